# revision 16
# baseline (speedup 1.0000x reference)
"""Trainium2 Bass kernel for GCN(1->8) + flatten + big regression matvec.

Model (reference):
    h = GCNConv(x[4096,1], edge_index[2,131072], W1[1,8], b1[8])   # [4096, 8]
    h = relu(h.reshape(-1))                                        # [32768]
    y = h @ Wr[32768, 4096] + br                                   # [4096]

Since x is [N,1] and W1 is [1,8], the GCN collapses to a per-node scalar
    s[d] = dinv[d] * sum_s C'[d, s] * u[s],   u = x * dinv,
    dinv = 1/sqrt(1 + indeg),   C' = edge-count matrix + I,
and h[d,k] = relu(s[d]*W1[k] + b1[k]).

Fast path (b1 == 0, the spec's fill): relu(s*W1_k) factorizes as
    relu(s*W1_k) = max(W1_k,0)*max(s,0) + max(-W1_k,0)*max(-s,0),
so the 32768-deep regression contraction collapses to one row per node:
    y = sum_n s_n * B[n,:] + br,
    B[n,:] = A+[n,:] if s_n >= 0 else -A-[n,:],
    A+/-[n,:] = sum_k max(+/-W1_k, 0) * Wr[8n+k, :]   (host weight fold).
Each core owns 512 nodes (grid columns 0..3 after the SPMD column
rotation): it runs the message passing for them (dense fp8 C' matmul),
then column 0 is served STATICALLY (A+/A- chunks stream in while the
sign pipeline runs, with relu(s)/relu(-s) coefficients) and columns 1..3
by sign-selected GPSIMD dma_gathers over A_all = [A+; -A-] (half the
HBM bytes of a static A+/A- load).  The A tiles are the stationary
(lhsT) matmul operand so the PE streams one output column per chunk.
Per-chunk partial sums go straight from PSUM to HBM; the host adds the
partials and br during the cross-core reduction.

Device index pipeline for the gathers:
  neg4096[p,c] = 4096*(agg[p,c] < 0)                         (DVE, f32)
  rhs[p,8c+g]  = neg4096[p,c] * mask[p,8c+g]                 (DVE x3)
  idx[m,t]     = sum_p perm[p,m]*rhs[p,t]                    (PE, f32)
  idxs         = int16(idx + wrapbase)                       (DVE)
where mask[p,t] = (p//16 == t%8), perm[p,m] = (p%16 == m%16), and
wrapbase[m,t] holds the A-row id of slot (m,t)'s node: this lands the
row-ids in dma_gather's "wrapped in 16 partitions, replicated per Q7
core" index layout (slot (q,t) holds the id for gather position 16t+q).
sign(s) == sign(agg) since dinv > 0, so the index pipeline races ahead
of s.

General path (b1 != 0): original row-parallel kernel over the full Wr.
"""

import numpy as np
import ml_dtypes

import concourse.bacc as bacc
import concourse.bass as bass
import concourse.mybir as mybir
import concourse.tile as tile
from concourse import library_config
from concourse.bass_utils import run_bass_kernel_spmd

N = 4096            # nodes
HID = 8             # GCN hidden dim
Y = 4096            # output dim
NCORES = 8
NPC = N // NCORES   # 512 nodes per core
WR_DT = mybir.dt.bfloat16
WR_NP = ml_dtypes.bfloat16

F32 = mybir.dt.float32
FP8 = mybir.dt.float8e4
BF16 = mybir.dt.bfloat16
I32 = mybir.dt.int32
I16 = mybir.dt.int16
AF = mybir.ActivationFunctionType
OP = mybir.AluOpType

CT_SPLIT = 4
# packed2 cols (int32 words): 0:96 packed x/indptr, then f32-bitcast consts:
# 96:128 mask, 128:256 perm, 256:288 wrapbase, 288:291 scalevec
IC_COLS = 96 + 32 + 128 + 32 + 3


def _build_kernel_fast(ct_bf16=False):
    """b1 == 0 path: folded-A matvec, static col 0 + sign-gather cols 1-3."""
    nc = bacc.Bacc("TRN2", target_bir_lowering=False, debug=False,
                   num_devices=NCORES)

    pk_d = nc.dram_tensor("packed", [128, IC_COLS], I32, kind="ExternalInput")
    ct_dt = BF16 if ct_bf16 else FP8
    ct_d = nc.dram_tensor("ct", [N, NPC], ct_dt, kind="ExternalInput")
    a_d = nc.dram_tensor("a_all", [2 * N, Y], WR_DT, kind="ExternalInput")
    a0_d = nc.dram_tensor("a0", [256, Y], WR_DT, kind="ExternalInput")
    y_ds = [nc.dram_tensor(f"y{i}", [128, 32], F32, kind="ExternalOutput")
            for i in range(4)]

    with tile.TileContext(nc) as tc:
        with (
            tc.tile_pool(name="small", bufs=1) as sp,
            tc.tile_pool(name="bmat", bufs=1) as wp,
        ):
            nc.gpsimd.load_library(library_config.mlp)

            # ---- DMAs: packed+consts, ct (4 src-chunks), static A0 ----
            pk_sb = sp.tile([128, IC_COLS], I32)
            nc.sync.dma_start(out=pk_sb[:], in_=pk_d[:])
            x_sb = pk_sb[:, 0:32].bitcast(F32)
            inda_sb = pk_sb[:, 32:64]
            indb_sb = pk_sb[:, 64:96]
            mask = pk_sb[:, 96:128].bitcast(F32)
            perm = pk_sb[:, 128:256].bitcast(F32)
            wrapbase = pk_sb[:, 256:288].bitcast(F32)
            scalevec = pk_sb[:, 288:291].bitcast(F32)

            ct_sb = sp.tile([128, 32 * NPC], ct_dt)
            ctv = ct_sb[:].rearrange("p (sc q) -> p sc q", q=NPC)
            ctd = ct_d[:].rearrange("(sc p) q -> p sc q", p=128)
            SC_PER = 32 // CT_SPLIT
            for h in range(CT_SPLIT):
                nc.sync.dma_start(out=ctv[:, SC_PER * h:SC_PER * (h + 1), :],
                                  in_=ctd[:, SC_PER * h:SC_PER * (h + 1), :])
            a0_tiles = [wp.tile([128, Y], WR_DT, name=f"a0{i}")
                        for i in range(2)]
            for i in range(2):
                nc.sync.dma_start(out=a0_tiles[i][:],
                                  in_=a0_d[128 * i:128 * (i + 1), :])

            # ---- deg -> dinv (Rsqrt + two Newton steps) ----
            degf_sb = sp.tile([128, 32], F32)
            degi_sb = sp.tile([128, 32], I32)
            nc.vector.tensor_tensor(out=degi_sb[:], in0=indb_sb,
                                    in1=inda_sb, op=OP.subtract)
            nc.vector.tensor_scalar_add(degi_sb[:], degi_sb[:], 1)
            nc.vector.tensor_copy(out=degf_sb[:], in_=degi_sb[:])
            sq_sb = sp.tile([128, 32], F32)
            nc.scalar.activation(sq_sb[:], degf_sb[:], AF.Sqrt)
            y0_sb = sp.tile([128, 32], F32)
            nc.vector.reciprocal(y0_sb[:], sq_sb[:])
            t_sb = sp.tile([128, 32], F32)
            dinv_sb = sp.tile([128, 32], F32)
            for cur, nxt in [(y0_sb, t_sb), (t_sb, dinv_sb)]:
                tmp_sb = sp.tile([128, 32], F32, name=f"nr_{nxt.tensor.name}")
                nc.vector.tensor_tensor(out=tmp_sb[:], in0=cur[:], in1=cur[:],
                                        op=OP.mult)
                nc.vector.tensor_tensor(out=tmp_sb[:], in0=tmp_sb[:],
                                        in1=degf_sb[:], op=OP.mult)
                nc.vector.tensor_scalar(out=tmp_sb[:], in0=tmp_sb[:],
                                        scalar1=-0.5, scalar2=1.5,
                                        op0=OP.mult, op1=OP.add)
                nc.vector.tensor_tensor(out=nxt[:], in0=cur[:], in1=tmp_sb[:],
                                        op=OP.mult)

            # ---- u = x*dinv, split into three scaled fp8 terms ----
            u_sb = sp.tile([128, 32], F32)
            nc.vector.tensor_tensor(out=u_sb[:], in0=x_sb, in1=dinv_sb[:],
                                    op=OP.mult)
            u2_sb = sp.tile([128, 96], FP8)
            u2v = u2_sb[:].rearrange("p (c three) -> p c three", three=3)
            res_sb = sp.tile([128, 32], F32)
            for term, scale in enumerate((1.0, 64.0, 4096.0)):
                scl_sb = sp.tile([128, 32], F32, name=f"scl{term}")
                if scale == 1.0:
                    src_ap = u_sb[:]
                else:
                    nc.vector.tensor_scalar_mul(scl_sb[:], u_sb[:]
                                                if term == 0 else res_sb[:],
                                                scale)
                    src_ap = scl_sb[:]
                nc.vector.tensor_copy(
                    out=u2v[:, :, term:term + 1],
                    in_=src_ap.rearrange("p (c one) -> p c one", one=1))
                if term < 2:
                    back_sb = sp.tile([128, 32], F32, name=f"back{term}")
                    nc.vector.tensor_copy(
                        out=back_sb[:].rearrange("p (c one) -> p c one",
                                                 one=1),
                        in_=u2v[:, :, term:term + 1])
                    if scale != 1.0:
                        nc.vector.tensor_scalar_mul(back_sb[:], back_sb[:],
                                                    1.0 / scale)
                    nc.vector.tensor_tensor(
                        out=res_sb[:],
                        in0=(u_sb[:] if term == 0 else res_sb[:]),
                        in1=back_sb[:], op=OP.subtract)

            with tc.tile_pool(name="psum_mp", bufs=1, space="PSUM") as pp1:
                # ---- agg = C'.u: sc-outer so matmuls chase the ct chunks
                agg_ps = [pp1.tile([128, 3], F32, name=f"aps{db}")
                          for db in range(4)]
                for sc in range(32):
                    for db in range(4):
                        nc.tensor.matmul(
                            out=agg_ps[db][:],
                            lhsT=ct_sb[:, NPC * sc + 128 * db:
                                       NPC * sc + 128 * (db + 1)],
                            rhs=u2_sb[:, 3 * sc:3 * sc + 3],
                            start=(sc == 0), stop=(sc == 31))

                # aggt[:, 3db:3db+3] = ps_db * (1, 1/64, 1/4096)
                aggt_sb = sp.tile([128, 12], F32)
                for db in range(4):
                    nc.vector.tensor_tensor(
                        out=aggt_sb[:, 3 * db:3 * db + 3],
                        in0=agg_ps[db][:], in1=scalevec, op=OP.mult)
                agg_sb = sp.tile([128, 4], F32)
                nc.vector.tensor_reduce(
                    out=agg_sb[:],
                    in_=aggt_sb[:].rearrange("p (db three) -> p db three",
                                             three=3),
                    axis=mybir.AxisListType.X, op=OP.add)

                # ---- gather indices (sign from agg; dinv > 0) ----
                neg_sb = sp.tile([128, 4], F32)
                nc.vector.tensor_scalar(out=neg_sb[:], in0=agg_sb[:],
                                        scalar1=0.0, scalar2=float(N),
                                        op0=OP.is_lt, op1=OP.mult)
                rhs_sb = sp.tile([128, 32], F32)
                for c in range(1, 4):
                    nc.vector.tensor_scalar(
                        out=rhs_sb[:, 8 * c:8 * (c + 1)],
                        in0=mask[:, 8 * c:8 * (c + 1)],
                        scalar1=neg_sb[:, c:c + 1], scalar2=None,
                        op0=OP.mult)
                nc.vector.memset(rhs_sb[:, 0:8], 0.0)
                idx_ps = pp1.tile([128, 32], F32, name="idxps")
                nc.tensor.matmul(out=idx_ps[:], lhsT=perm, rhs=rhs_sb[:],
                                 start=True, stop=True)
                idxs_sb = sp.tile([128, 32], I16)
                nc.vector.tensor_tensor(out=idxs_sb[:], in0=idx_ps[:],
                                        in1=wrapbase, op=OP.add)

                # s = dinv_own * agg; coeff = s (A- rows are pre-negated)
                co_sb = sp.tile([128, 4], BF16)
                nc.vector.tensor_tensor(out=co_sb[:], in0=agg_sb[:],
                                        in1=dinv_sb[:, 0:4], op=OP.mult)
                # relu coeffs for the static column 0
                spm_sb = sp.tile([128, 2], BF16)
                nc.vector.tensor_scalar_max(spm_sb[:, 0:1], co_sb[:, 0:1],
                                            0.0)
                nc.vector.tensor_scalar(out=spm_sb[:, 1:2],
                                        in0=co_sb[:, 0:1],
                                        scalar1=-1.0, scalar2=0.0,
                                        op0=OP.mult, op1=OP.max)

            # ---- gather own columns 1..3 (128 rows each) ----
            b_tiles = [wp.tile([128, Y], WR_DT, name=f"b{g}")
                       for g in range(1, 4)]
            for g in range(1, 4):
                nc.gpsimd.dma_gather(
                    b_tiles[g - 1][:].rearrange("p (one e) -> p one e",
                                                one=1),
                    a_d[:], idxs_sb[:, 8 * g:8 * (g + 1)], 128, 128, Y)

            # ---- partial sums straight to DRAM; host reduces + adds br ----
            with tc.tile_pool(name="psum_y", bufs=1, space="PSUM") as pp:
                y_ps = [pp.tile([128, 32], F32, name=f"yps{i}")
                        for i in range(4)]
                y_sbs = [sp.tile([128, 32], F32, name=f"ysb{i}")
                         for i in range(4)]
                # static column 0: A0+ with relu(s), A0- with relu(-s)
                for cc in range(32):
                    nc.tensor.matmul(
                        out=y_ps[0][:, cc:cc + 1],
                        lhsT=a0_tiles[0][:, 128 * cc:128 * (cc + 1)],
                        rhs=spm_sb[:, 0:1],
                        start=True, stop=False, skip_group_check=True)
                    nc.tensor.matmul(
                        out=y_ps[0][:, cc:cc + 1],
                        lhsT=a0_tiles[1][:, 128 * cc:128 * (cc + 1)],
                        rhs=spm_sb[:, 1:2],
                        start=False, stop=True, skip_group_check=True)
                nc.vector.tensor_copy(out=y_sbs[0][:], in_=y_ps[0][:])
                nc.sync.dma_start(out=y_ds[0][:], in_=y_sbs[0][:])
                for g in range(1, 4):
                    for cc in range(32):
                        nc.tensor.matmul(
                            out=y_ps[g][:, cc:cc + 1],
                            lhsT=b_tiles[g - 1][:, 128 * cc:128 * (cc + 1)],
                            rhs=co_sb[:, g:g + 1],
                            start=True, stop=True,
                            skip_group_check=True)
                    if g == 2:
                        nc.scalar.copy(out=y_sbs[g][:], in_=y_ps[g][:])
                    else:
                        nc.vector.tensor_copy(out=y_sbs[g][:],
                                              in_=y_ps[g][:])
                    nc.sync.dma_start(out=y_ds[g][:], in_=y_sbs[g][:])

    nc.compile()
    return nc


def _mp_subgraph(nc, sp, pp, pk_d, ct_d, ct_dt):
    """Message passing for the general path: packed x/indptr + dense C'
    -> s [128, 4] fp32.  Returns (s_sb, dinv_sb)."""
    pk_sb = sp.tile([128, 96], I32)
    nc.sync.dma_start(out=pk_sb[:], in_=pk_d[:])
    x_sb = pk_sb[:, 0:32].bitcast(F32)
    inda_sb = pk_sb[:, 32:64]
    indb_sb = pk_sb[:, 64:96]
    ct_sb = sp.tile([128, 32 * NPC], ct_dt)
    nc.sync.dma_start(
        out=ct_sb[:].rearrange("p (sc q) -> p sc q", q=NPC),
        in_=ct_d[:].rearrange("(sc p) q -> p sc q", p=128))

    degf_sb = sp.tile([128, 32], F32)
    degi_sb = sp.tile([128, 32], I32)
    nc.vector.tensor_tensor(out=degi_sb[:], in0=indb_sb,
                            in1=inda_sb, op=OP.subtract)
    nc.vector.tensor_scalar_add(degi_sb[:], degi_sb[:], 1)
    nc.vector.tensor_copy(out=degf_sb[:], in_=degi_sb[:])
    sq_sb = sp.tile([128, 32], F32)
    nc.scalar.activation(sq_sb[:], degf_sb[:], AF.Sqrt)
    y0_sb = sp.tile([128, 32], F32)
    nc.vector.reciprocal(y0_sb[:], sq_sb[:])
    t_sb = sp.tile([128, 32], F32)
    dinv_sb = sp.tile([128, 32], F32)
    for cur, nxt in [(y0_sb, t_sb), (t_sb, dinv_sb)]:
        tmp_sb = sp.tile([128, 32], F32, name=f"nr_{nxt.tensor.name}")
        nc.vector.tensor_tensor(out=tmp_sb[:], in0=cur[:], in1=cur[:],
                                op=OP.mult)
        nc.vector.tensor_tensor(out=tmp_sb[:], in0=tmp_sb[:],
                                in1=degf_sb[:], op=OP.mult)
        nc.vector.tensor_scalar(out=tmp_sb[:], in0=tmp_sb[:],
                                scalar1=-0.5, scalar2=1.5,
                                op0=OP.mult, op1=OP.add)
        nc.vector.tensor_tensor(out=nxt[:], in0=cur[:], in1=tmp_sb[:],
                                op=OP.mult)

    u_sb = sp.tile([128, 32], F32)
    nc.vector.tensor_tensor(out=u_sb[:], in0=x_sb, in1=dinv_sb[:],
                            op=OP.mult)
    u2_sb = sp.tile([128, 96], FP8)
    u2v = u2_sb[:].rearrange("p (c three) -> p c three", three=3)
    res_sb = sp.tile([128, 32], F32)
    for term, scale in enumerate((1.0, 64.0, 4096.0)):
        scl_sb = sp.tile([128, 32], F32, name=f"scl{term}")
        if scale == 1.0:
            src_ap = u_sb[:]
        else:
            nc.vector.tensor_scalar_mul(scl_sb[:], u_sb[:]
                                        if term == 0 else res_sb[:],
                                        scale)
            src_ap = scl_sb[:]
        nc.vector.tensor_copy(
            out=u2v[:, :, term:term + 1],
            in_=src_ap.rearrange("p (c one) -> p c one", one=1))
        if term < 2:
            back_sb = sp.tile([128, 32], F32, name=f"back{term}")
            nc.vector.tensor_copy(
                out=back_sb[:].rearrange("p (c one) -> p c one", one=1),
                in_=u2v[:, :, term:term + 1])
            if scale != 1.0:
                nc.vector.tensor_scalar_mul(back_sb[:], back_sb[:],
                                            1.0 / scale)
            nc.vector.tensor_tensor(
                out=res_sb[:], in0=(u_sb[:] if term == 0 else res_sb[:]),
                in1=back_sb[:], op=OP.subtract)

    agg_ps = [pp.tile([128, 3], F32, name=f"aps{db}") for db in range(4)]
    for db in range(4):
        for sc in range(32):
            nc.tensor.matmul(
                out=agg_ps[db][:],
                lhsT=ct_sb[:, NPC * sc + 128 * db:NPC * sc + 128 * (db + 1)],
                rhs=u2_sb[:, 3 * sc:3 * sc + 3],
                start=(sc == 0), stop=(sc == 31))
    aggt_sb = sp.tile([128, 12], F32)
    for db in range(4):
        nc.vector.tensor_copy(out=aggt_sb[:, 3 * db:3 * db + 3],
                              in_=agg_ps[db][:])
    agg_sb = sp.tile([128, 4], F32)
    av = aggt_sb[:].rearrange("p (db three) -> p db three", three=3)
    nc.vector.tensor_scalar_mul(av[:, :, 1:2], av[:, :, 1:2], 1.0 / 64)
    nc.vector.tensor_scalar_mul(av[:, :, 2:3], av[:, :, 2:3], 1.0 / 4096)
    nc.vector.tensor_reduce(out=agg_sb[:], in_=av,
                            axis=mybir.AxisListType.X, op=OP.add)

    s_sb = sp.tile([128, 4], F32)
    nc.vector.tensor_tensor(out=s_sb[:], in0=agg_sb[:],
                            in1=dinv_sb[:, 0:4], op=OP.mult)
    return s_sb, dinv_sb


def _build_kernel_general(ct_bf16=False):
    """b1 != 0 fallback: original kernel, full Wr row-parallel matvec."""
    nc = bacc.Bacc("TRN2", target_bir_lowering=False, debug=False,
                   num_devices=NCORES)

    pk_d = nc.dram_tensor("packed", [128, 96], I32, kind="ExternalInput")
    ct_dt = BF16 if ct_bf16 else FP8
    ct_d = nc.dram_tensor("ct", [N, NPC], ct_dt, kind="ExternalInput")
    wb_d = nc.dram_tensor("w1b1", [1, 2 * HID], F32, kind="ExternalInput")
    bias_d = nc.dram_tensor("bias", [1, Y], F32, kind="ExternalInput")
    wr_d = nc.dram_tensor("wr", [8 * NPC, Y], WR_DT, kind="ExternalInput")
    y_d = nc.dram_tensor("y", [1, Y], F32, kind="ExternalOutput")

    with tile.TileContext(nc) as tc:
        with (
            tc.tile_pool(name="small", bufs=1) as sp,
            tc.tile_pool(name="wr", bufs=1) as wp,
            tc.tile_pool(name="psum", bufs=1, space="PSUM") as pp,
        ):
            wbrow = sp.tile([1, 2 * HID], F32)
            nc.sync.dma_start(out=wbrow[:], in_=wb_d[:])
            w1row = wbrow[:, 0:HID]
            b1row = wbrow[:, HID:2 * HID]
            bias_sb = sp.tile([1, Y], F32)
            nc.sync.dma_start(out=bias_sb[:], in_=bias_d[:])

            s_sb, _ = _mp_subgraph(nc, sp, pp, pk_d, ct_d, ct_dt)

            ones_sb = sp.tile([1, 128], F32)
            nc.vector.memset(ones_sb[:], 1.0)
            wb_ps = pp.tile([128, 2 * HID], F32, name="ps4")
            nc.tensor.matmul(out=wb_ps[:, 0:HID], lhsT=ones_sb[:],
                             rhs=w1row, start=True, stop=True)
            nc.tensor.matmul(out=wb_ps[:, HID:2 * HID], lhsT=ones_sb[:],
                             rhs=b1row, start=True, stop=True)
            wb_sb = sp.tile([128, 2 * HID], F32)
            nc.vector.tensor_copy(out=wb_sb[:], in_=wb_ps[:])

            h_sb = sp.tile([128, 4 * HID], BF16)
            for kk in range(HID):
                nc.vector.tensor_scalar(
                    out=h_sb[:, 4 * kk:4 * kk + 4], in0=s_sb[:],
                    scalar1=wb_sb[:, kk:kk + 1],
                    scalar2=wb_sb[:, HID + kk:HID + kk + 1],
                    op0=OP.mult, op1=OP.add)
            nc.vector.tensor_scalar_max(h_sb[:], h_sb[:], 0.0)

            # alias the MP agg banks (aps0-3) and wb bank (ps4): the PSUM
            # pool dedupes tiles by name and only 8 banks exist.  Bias is
            # added during the PSUM->SBUF copy (a DVE preload would be lost:
            # only TensorE matmuls set the has_written accumulate bits).
            y_ps = [pp.tile([1, 512], F32,
                            name=(f"aps{bk}" if bk < 4 else f"ps{bk}"))
                    for bk in range(8)]
            for t in range(32):
                wr_sb = wp.tile([128, Y], WR_DT, name=f"wr{t % 12}")
                nc.sync.dma_start(out=wr_sb[:],
                                  in_=wr_d[128 * t:128 * (t + 1), :])
                kk, c = t // 4, t % 4
                hcol = h_sb[:, 4 * kk + c:4 * kk + c + 1]
                for bk in range(8):
                    nc.tensor.matmul(out=y_ps[bk][:], lhsT=hcol,
                                     rhs=wr_sb[:, 512 * bk:512 * (bk + 1)],
                                     start=(t == 0), stop=(t == 31),
                                     skip_group_check=True)

            y_sb = sp.tile([1, Y], F32)
            for bk in range(8):
                nc.vector.tensor_tensor(
                    out=y_sb[:, 512 * bk:512 * (bk + 1)],
                    in0=y_ps[bk][:],
                    in1=bias_sb[:, 512 * bk:512 * (bk + 1)], op=OP.add)
            nc.sync.dma_start(out=y_d[:], in_=y_sb[:])

    nc.compile()
    return nc


_NC_CACHE = {}


def _get_nc(kind, ct_bf16=False):
    key = (kind, ct_bf16)
    if key not in _NC_CACHE:
        build = _build_kernel_fast if kind == "fast" else _build_kernel_general
        _NC_CACHE[key] = build(ct_bf16)
    return _NC_CACHE[key]


def _graph_prep(x, edge_index):
    """Per-core packed x/indptr arrays and dense count matrices."""
    x = np.ascontiguousarray(x, dtype=np.float32).reshape(N)
    src = np.asarray(edge_index[0], dtype=np.int64)
    dst = np.asarray(edge_index[1], dtype=np.int64)

    indeg = np.bincount(dst, minlength=N)
    indptr = np.zeros(N + 1, dtype=np.int32)
    np.cumsum(indeg, out=indptr[1:])

    packs, cts, any_bf16 = [], [], False
    p = np.arange(128)[:, None]
    for k in range(NCORES):
        rot = (np.arange(32) + 4 * k) % 32          # column rotation
        g = 128 * rot[None, :] + p                  # [128, 32] global node ids

        mask = (dst >= NPC * k) & (dst < NPC * (k + 1))
        ck = np.zeros((NPC, N), dtype=np.float32)
        np.add.at(ck, (dst[mask] - NPC * k, src[mask]), 1.0)
        ck[np.arange(NPC), NPC * k + np.arange(NPC)] += 1.0
        # counts <= 8 are exact in fp8e4m3; fall back to bf16 otherwise
        any_bf16 = any_bf16 or bool(ck.max() > 8)
        srcperm = g.T.reshape(-1)                   # [(sc i)] -> global node
        cts.append((ck, srcperm))

        packed = np.concatenate([
            x[g].astype(np.float32).view(np.int32),
            indptr[g].astype(np.int32),
            indptr[g + 1].astype(np.int32)], axis=1)
        packs.append(np.ascontiguousarray(packed))
    ct_np = ml_dtypes.bfloat16 if any_bf16 else ml_dtypes.float8_e4m3
    cts = [np.ascontiguousarray(ck[:, srcperm].T).astype(ct_np)
           for ck, srcperm in cts]
    return packs, cts, any_bf16


def _host_prep_fast(x, edge_index, W1, b1, Wr, br):
    """Graph layout plus the W1->A weight fold (b1 == 0 only)."""
    packs, cts, any_bf16 = _graph_prep(x, edge_index)
    W1v = np.ascontiguousarray(W1, dtype=np.float32).reshape(HID)
    Wr3 = np.ascontiguousarray(Wr, dtype=np.float32).reshape(N, HID, Y)

    # A+/-[n, :] = sum_k max(+/-W1_k, 0) * Wr[8n+k, :]
    w1p = np.maximum(W1v, 0.0)
    w1m = np.maximum(-W1v, 0.0)
    ap = np.tensordot(Wr3, w1p, axes=([1], [0]))   # [N, Y]
    am = np.tensordot(Wr3, w1m, axes=([1], [0]))   # [N, Y]
    a_all = np.ascontiguousarray(
        np.concatenate([ap, -am], axis=0)).astype(WR_NP)  # [2N, Y]

    p = np.arange(128)[:, None]
    mask = (p // 16 == np.arange(32)[None, :] % 8).astype(np.float32)
    mask[:, 0:8] = 0.0                              # column 0 is static
    perm = (p % 16 == np.arange(128)[None, :] % 16).astype(np.float32)
    scalevec = np.tile(np.array([[1.0, 1.0 / 64, 1.0 / 4096]],
                                dtype=np.float32), (128, 1))

    t = np.arange(32)[None, :]
    in_maps = []
    for k in range(NCORES):
        # wrapbase[m, t] = own-node A-row id of gather slot (m, t)
        wrapbase = (512 * k + 128 * (t // 8) + 16 * (t % 8)
                    + p % 16).astype(np.float32)
        consts = np.concatenate([mask, perm, wrapbase, scalevec], axis=1)
        packed2 = np.concatenate([packs[k], consts.view(np.int32)], axis=1)
        # static column 0: rows = A+ then A- for nodes 512k .. 512k+127
        # (coeffs are relu(s) and relu(-s), both nonnegative)
        a0 = np.concatenate([ap[512 * k:512 * k + 128],
                             am[512 * k:512 * k + 128]], axis=0)
        in_maps.append({
            "packed": np.ascontiguousarray(packed2),
            "ct": cts[k],
            "a_all": a_all,
            "a0": np.ascontiguousarray(a0.astype(WR_NP)),
        })
    return in_maps, any_bf16


def _host_prep_general(x, edge_index, W1, b1, Wr, br):
    packs, cts, any_bf16 = _graph_prep(x, edge_index)
    W1v = np.ascontiguousarray(W1, dtype=np.float32).reshape(1, HID)
    b1v = np.ascontiguousarray(b1, dtype=np.float32).reshape(1, HID)
    brv = np.ascontiguousarray(br, dtype=np.float32).reshape(1, Y)
    Wr3 = np.ascontiguousarray(Wr, dtype=np.float32).reshape(N, HID, Y)

    in_maps = []
    for k in range(NCORES):
        wr_core = np.ascontiguousarray(
            Wr3[NPC * k:NPC * (k + 1)].transpose(1, 0, 2).reshape(8 * NPC, Y),
            dtype=np.float32).astype(WR_NP)
        in_maps.append({
            "packed": packs[k],
            "ct": cts[k],
            "w1b1": np.concatenate([W1v, b1v], axis=1),
            "bias": brv if k == 0 else np.zeros((1, Y), dtype=np.float32),
            "wr": wr_core,
        })
    return in_maps, any_bf16


def _run_fast(x, edge_index, W1, b1, Wr, br, _trace):
    in_maps, ct_bf16 = _host_prep_fast(x, edge_index, W1, b1, Wr, br)
    nc = _get_nc("fast", ct_bf16)
    try:
        res = run_bass_kernel_spmd(nc, in_maps, list(range(NCORES)),
                                   trace=_trace)
    except Exception:
        # one retry: recovers from transiently-poisoned device state
        res = run_bass_kernel_spmd(nc, in_maps, list(range(NCORES)),
                                   trace=_trace)
    y = np.asarray(br, dtype=np.float64).reshape(Y).copy()
    for k in range(NCORES):
        for i in range(4):
            yk = np.asarray(res.results[k][f"y{i}"]).astype(np.float64)
            y += yk.reshape(128, 32).T.reshape(Y)
    return y, res


def _run_general(x, edge_index, W1, b1, Wr, br, _trace):
    in_maps, ct_bf16 = _host_prep_general(x, edge_index, W1, b1, Wr, br)
    nc = _get_nc("general", ct_bf16)
    try:
        res = run_bass_kernel_spmd(nc, in_maps, list(range(NCORES)),
                                   trace=_trace)
    except Exception:
        res = run_bass_kernel_spmd(nc, in_maps, list(range(NCORES)),
                                   trace=_trace)
    y = np.zeros(Y, dtype=np.float64)
    for k in range(NCORES):
        y += np.asarray(res.results[k]["y"]).reshape(Y).astype(np.float64)
    return y, res


def kernel(x, edge_index, W1, b1, Wr, br, _trace=False):
    fast = not np.asarray(b1, dtype=np.float32).any()
    y = res = None
    if fast:
        try:
            y, res = _run_fast(x, edge_index, W1, b1, Wr, br, _trace)
        except Exception:
            y = None  # e.g. gather/gpsimd library unavailable: use fallback
    if y is None:
        y, res = _run_general(x, edge_index, W1, b1, Wr, br, _trace)
    out = y.astype(np.float32)
    if _trace:
        return out, res
    return out


# revision 18
# speedup vs baseline: 1.0105x; 1.0105x over previous
"""Trainium2 Bass kernel for GCN(1->8) + flatten + big regression matvec.

Model (reference):
    h = GCNConv(x[4096,1], edge_index[2,131072], W1[1,8], b1[8])   # [4096, 8]
    h = relu(h.reshape(-1))                                        # [32768]
    y = h @ Wr[32768, 4096] + br                                   # [4096]

Since x is [N,1] and W1 is [1,8], the GCN collapses to a per-node scalar
    s[d] = dinv[d] * sum_s C'[d, s] * u[s],   u = x * dinv,
    dinv = 1/sqrt(1 + indeg),   C' = edge-count matrix + I,
and h[d,k] = relu(s[d]*W1[k] + b1[k]).

Fast path (b1 == 0, the spec's fill): relu(s*W1_k) factorizes as
    relu(s*W1_k) = max(W1_k,0)*max(s,0) + max(-W1_k,0)*max(-s,0),
so the 32768-deep regression contraction collapses to one row per node:
    y = sum_n s_n * B[n,:] + br,
    B[n,:] = A+[n,:] if s_n >= 0 else -A-[n,:],
    A+/-[n,:] = sum_k max(+/-W1_k, 0) * Wr[8n+k, :]   (host weight fold).
Each core owns 512 nodes (grid columns 0..3 after the SPMD column
rotation): it runs the message passing for them (dense fp8 C' matmul),
then column 0 is served STATICALLY (A+/A- chunks stream in while the
sign pipeline runs, with relu(s)/relu(-s) coefficients) and columns 1..3
by sign-selected GPSIMD dma_gathers over A_all = [A+; -A-] (half the
HBM bytes of a static A+/A- load).  The A tiles are the stationary
(lhsT) matmul operand so the PE streams one output column per chunk.
Per-chunk partial sums go straight from PSUM to HBM; the host adds the
partials and br during the cross-core reduction.

Device index pipeline for the gathers:
  neg4096[p,c] = 4096*(agg[p,c] < 0)                         (DVE, f32)
  rhs[p,8c+g]  = neg4096[p,c] * mask[p,8c+g]                 (DVE x3)
  idx[m,t]     = sum_p perm[p,m]*rhs[p,t]                    (PE, f32)
  idxs         = int16(idx + wrapbase)                       (DVE)
where mask[p,t] = (p//16 == t%8), perm[p,m] = (p%16 == m%16), and
wrapbase[m,t] holds the A-row id of slot (m,t)'s node: this lands the
row-ids in dma_gather's "wrapped in 16 partitions, replicated per Q7
core" index layout (slot (q,t) holds the id for gather position 16t+q).
sign(s) == sign(agg) since dinv > 0, so the index pipeline races ahead
of s.

General path (b1 != 0): original row-parallel kernel over the full Wr.
"""

import numpy as np
import ml_dtypes

import concourse.bacc as bacc
import concourse.bass as bass
import concourse.mybir as mybir
import concourse.tile as tile
from concourse import library_config
from concourse.bass_utils import run_bass_kernel_spmd

N = 4096            # nodes
HID = 8             # GCN hidden dim
Y = 4096            # output dim
NCORES = 8
NPC = N // NCORES   # 512 nodes per core
WR_DT = mybir.dt.bfloat16
WR_NP = ml_dtypes.bfloat16

F32 = mybir.dt.float32
FP8 = mybir.dt.float8e4
BF16 = mybir.dt.bfloat16
I32 = mybir.dt.int32
I16 = mybir.dt.int16
AF = mybir.ActivationFunctionType
OP = mybir.AluOpType

CT_SPLIT = 4
# packed2 cols (int32 words): 0:96 packed x/indptr, then f32-bitcast consts:
# 96:128 mask, 128:256 perm, 256:288 wrapbase, 288:291 scalevec
IC_COLS = 96 + 32 + 128 + 32 + 3


def _build_kernel_fast(ct_bf16=False):
    """b1 == 0 path: folded-A matvec, static col 0 + sign-gather cols 1-3."""
    nc = bacc.Bacc("TRN2", target_bir_lowering=False, debug=False,
                   num_devices=NCORES)

    pk_d = nc.dram_tensor("packed", [128, IC_COLS], I32, kind="ExternalInput")
    ct_dt = BF16 if ct_bf16 else FP8
    ct_d = nc.dram_tensor("ct", [N, NPC], ct_dt, kind="ExternalInput")
    a_d = nc.dram_tensor("a_all", [2 * N, Y], WR_DT, kind="ExternalInput")
    a0_d = nc.dram_tensor("a0", [256, Y], WR_DT, kind="ExternalInput")
    y_ds = [nc.dram_tensor(f"y{i}", [128, 32], F32, kind="ExternalOutput")
            for i in range(4)]

    with tile.TileContext(nc) as tc:
        with (
            tc.tile_pool(name="small", bufs=1) as sp,
            tc.tile_pool(name="bmat", bufs=1) as wp,
        ):
            nc.gpsimd.load_library(library_config.mlp)

            # ---- DMAs: ct (big chunk first hides the HWDGE pipeline so the
            # DMA engines never idle), then packed+consts, then static A0 ----
            pk_sb = sp.tile([128, IC_COLS], I32)
            x_sb = pk_sb[:, 0:32].bitcast(F32)
            inda_sb = pk_sb[:, 32:64]
            indb_sb = pk_sb[:, 64:96]
            mask = pk_sb[:, 96:128].bitcast(F32)
            perm = pk_sb[:, 128:256].bitcast(F32)
            wrapbase = pk_sb[:, 256:288].bitcast(F32)
            scalevec = pk_sb[:, 288:291].bitcast(F32)

            ct_sb = sp.tile([128, 32 * NPC], ct_dt)
            ctv = ct_sb[:].rearrange("p (sc q) -> p sc q", q=NPC)
            ctd = ct_d[:].rearrange("(sc p) q -> p sc q", p=128)
            # first ct chunk sized to cover the HWDGE pipeline of the packed
            # DMA behind it (zero DMA-engine gaps); packed lands early so the
            # ~2.9us DVE dinv/u chain overlaps the remaining ct chunks; the
            # tiny last chunk keeps the post-ct matmul tail short
            ct_bounds = [0, 5, 16, 26, 30, 32]
            nc.sync.dma_start(out=ctv[:, 0:5, :], in_=ctd[:, 0:5, :])
            nc.sync.dma_start(out=pk_sb[:], in_=pk_d[:])
            for h in range(1, len(ct_bounds) - 1):
                lo, hi = ct_bounds[h], ct_bounds[h + 1]
                nc.sync.dma_start(out=ctv[:, lo:hi, :], in_=ctd[:, lo:hi, :])
            a0_tiles = [wp.tile([128, Y], WR_DT, name=f"a0{i}")
                        for i in range(2)]
            for i in range(2):
                nc.sync.dma_start(out=a0_tiles[i][:],
                                  in_=a0_d[128 * i:128 * (i + 1), :])

            # ---- deg -> dinv (Rsqrt + two Newton steps) ----
            degf_sb = sp.tile([128, 32], F32)
            degi_sb = sp.tile([128, 32], I32)
            nc.vector.tensor_tensor(out=degi_sb[:], in0=indb_sb,
                                    in1=inda_sb, op=OP.subtract)
            nc.vector.tensor_scalar_add(degi_sb[:], degi_sb[:], 1)
            nc.vector.tensor_copy(out=degf_sb[:], in_=degi_sb[:])
            sq_sb = sp.tile([128, 32], F32)
            nc.scalar.activation(sq_sb[:], degf_sb[:], AF.Sqrt)
            y0_sb = sp.tile([128, 32], F32)
            nc.vector.reciprocal(y0_sb[:], sq_sb[:])
            t_sb = sp.tile([128, 32], F32)
            dinv_sb = sp.tile([128, 32], F32)
            for cur, nxt in [(y0_sb, t_sb), (t_sb, dinv_sb)]:
                tmp_sb = sp.tile([128, 32], F32, name=f"nr_{nxt.tensor.name}")
                nc.vector.tensor_tensor(out=tmp_sb[:], in0=cur[:], in1=cur[:],
                                        op=OP.mult)
                nc.vector.tensor_tensor(out=tmp_sb[:], in0=tmp_sb[:],
                                        in1=degf_sb[:], op=OP.mult)
                nc.vector.tensor_scalar(out=tmp_sb[:], in0=tmp_sb[:],
                                        scalar1=-0.5, scalar2=1.5,
                                        op0=OP.mult, op1=OP.add)
                nc.vector.tensor_tensor(out=nxt[:], in0=cur[:], in1=tmp_sb[:],
                                        op=OP.mult)

            # ---- u = x*dinv, split into three scaled fp8 terms ----
            u_sb = sp.tile([128, 32], F32)
            nc.vector.tensor_tensor(out=u_sb[:], in0=x_sb, in1=dinv_sb[:],
                                    op=OP.mult)
            u2_sb = sp.tile([128, 96], FP8)
            u2v = u2_sb[:].rearrange("p (c three) -> p c three", three=3)
            res_sb = sp.tile([128, 32], F32)
            for term, scale in enumerate((1.0, 64.0, 4096.0)):
                scl_sb = sp.tile([128, 32], F32, name=f"scl{term}")
                if scale == 1.0:
                    src_ap = u_sb[:]
                else:
                    nc.vector.tensor_scalar_mul(scl_sb[:], u_sb[:]
                                                if term == 0 else res_sb[:],
                                                scale)
                    src_ap = scl_sb[:]
                nc.vector.tensor_copy(
                    out=u2v[:, :, term:term + 1],
                    in_=src_ap.rearrange("p (c one) -> p c one", one=1))
                if term < 2:
                    back_sb = sp.tile([128, 32], F32, name=f"back{term}")
                    nc.vector.tensor_copy(
                        out=back_sb[:].rearrange("p (c one) -> p c one",
                                                 one=1),
                        in_=u2v[:, :, term:term + 1])
                    if scale != 1.0:
                        nc.vector.tensor_scalar_mul(back_sb[:], back_sb[:],
                                                    1.0 / scale)
                    nc.vector.tensor_tensor(
                        out=res_sb[:],
                        in0=(u_sb[:] if term == 0 else res_sb[:]),
                        in1=back_sb[:], op=OP.subtract)

            with tc.tile_pool(name="psum_mp", bufs=1, space="PSUM") as pp1:
                # ---- agg = C'.u: sc-outer so matmuls chase the ct chunks
                agg_ps = [pp1.tile([128, 3], F32, name=f"aps{db}")
                          for db in range(4)]
                for sc in range(32):
                    for db in range(4):
                        nc.tensor.matmul(
                            out=agg_ps[db][:],
                            lhsT=ct_sb[:, NPC * sc + 128 * db:
                                       NPC * sc + 128 * (db + 1)],
                            rhs=u2_sb[:, 3 * sc:3 * sc + 3],
                            start=(sc == 0), stop=(sc == 31))

                # aggt[:, 3db:3db+3] = ps_db * (1, 1/64, 1/4096)
                aggt_sb = sp.tile([128, 12], F32)
                for db in range(4):
                    nc.vector.tensor_tensor(
                        out=aggt_sb[:, 3 * db:3 * db + 3],
                        in0=agg_ps[db][:], in1=scalevec, op=OP.mult)
                agg_sb = sp.tile([128, 4], F32)
                nc.vector.tensor_reduce(
                    out=agg_sb[:],
                    in_=aggt_sb[:].rearrange("p (db three) -> p db three",
                                             three=3),
                    axis=mybir.AxisListType.X, op=OP.add)

                # ---- gather indices (sign from agg; dinv > 0) ----
                neg_sb = sp.tile([128, 4], F32)
                nc.vector.tensor_scalar(out=neg_sb[:], in0=agg_sb[:],
                                        scalar1=0.0, scalar2=float(N),
                                        op0=OP.is_lt, op1=OP.mult)
                rhs_sb = sp.tile([128, 32], F32)
                for c in range(1, 4):
                    nc.vector.tensor_scalar(
                        out=rhs_sb[:, 8 * c:8 * (c + 1)],
                        in0=mask[:, 8 * c:8 * (c + 1)],
                        scalar1=neg_sb[:, c:c + 1], scalar2=None,
                        op0=OP.mult)
                nc.vector.memset(rhs_sb[:, 0:8], 0.0)
                idx_ps = pp1.tile([128, 32], F32, name="idxps")
                nc.tensor.matmul(out=idx_ps[:], lhsT=perm, rhs=rhs_sb[:],
                                 start=True, stop=True)
                idxs_sb = sp.tile([128, 32], I16)
                nc.vector.tensor_tensor(out=idxs_sb[:], in0=idx_ps[:],
                                        in1=wrapbase, op=OP.add)

                # s = dinv_own * agg; coeff = s (A- rows are pre-negated)
                co_sb = sp.tile([128, 4], BF16)
                nc.vector.tensor_tensor(out=co_sb[:], in0=agg_sb[:],
                                        in1=dinv_sb[:, 0:4], op=OP.mult)
                # relu coeffs for the static column 0
                spm_sb = sp.tile([128, 2], BF16)
                nc.vector.tensor_scalar_max(spm_sb[:, 0:1], co_sb[:, 0:1],
                                            0.0)
                nc.vector.tensor_scalar(out=spm_sb[:, 1:2],
                                        in0=co_sb[:, 0:1],
                                        scalar1=-1.0, scalar2=0.0,
                                        op0=OP.mult, op1=OP.max)

            # ---- gather own columns 1..3 (128 rows each) ----
            b_tiles = [wp.tile([128, Y], WR_DT, name=f"b{g}")
                       for g in range(1, 4)]
            for g in range(1, 4):
                nc.gpsimd.dma_gather(
                    b_tiles[g - 1][:].rearrange("p (one e) -> p one e",
                                                one=1),
                    a_d[:], idxs_sb[:, 8 * g:8 * (g + 1)], 128, 128, Y)

            # ---- partial sums straight to DRAM; host reduces + adds br ----
            with tc.tile_pool(name="psum_y", bufs=1, space="PSUM") as pp:
                y_ps = [pp.tile([128, 32], F32, name=f"yps{i}")
                        for i in range(4)]
                y_sbs = [sp.tile([128, 32], F32, name=f"ysb{i}")
                         for i in range(4)]
                # static column 0: A0+ with relu(s), A0- with relu(-s)
                for cc in range(32):
                    nc.tensor.matmul(
                        out=y_ps[0][:, cc:cc + 1],
                        lhsT=a0_tiles[0][:, 128 * cc:128 * (cc + 1)],
                        rhs=spm_sb[:, 0:1],
                        start=True, stop=False, skip_group_check=True)
                    nc.tensor.matmul(
                        out=y_ps[0][:, cc:cc + 1],
                        lhsT=a0_tiles[1][:, 128 * cc:128 * (cc + 1)],
                        rhs=spm_sb[:, 1:2],
                        start=False, stop=True, skip_group_check=True)
                nc.vector.tensor_copy(out=y_sbs[0][:], in_=y_ps[0][:])
                nc.sync.dma_start(out=y_ds[0][:], in_=y_sbs[0][:])
                for g in range(1, 4):
                    for cc in range(32):
                        nc.tensor.matmul(
                            out=y_ps[g][:, cc:cc + 1],
                            lhsT=b_tiles[g - 1][:, 128 * cc:128 * (cc + 1)],
                            rhs=co_sb[:, g:g + 1],
                            start=True, stop=True,
                            skip_group_check=True)
                    if g == 2:
                        nc.scalar.copy(out=y_sbs[g][:], in_=y_ps[g][:])
                    else:
                        nc.vector.tensor_copy(out=y_sbs[g][:],
                                              in_=y_ps[g][:])
                    nc.sync.dma_start(out=y_ds[g][:], in_=y_sbs[g][:])

    nc.compile()
    return nc


def _mp_subgraph(nc, sp, pp, pk_d, ct_d, ct_dt):
    """Message passing for the general path: packed x/indptr + dense C'
    -> s [128, 4] fp32.  Returns (s_sb, dinv_sb)."""
    pk_sb = sp.tile([128, 96], I32)
    nc.sync.dma_start(out=pk_sb[:], in_=pk_d[:])
    x_sb = pk_sb[:, 0:32].bitcast(F32)
    inda_sb = pk_sb[:, 32:64]
    indb_sb = pk_sb[:, 64:96]
    ct_sb = sp.tile([128, 32 * NPC], ct_dt)
    nc.sync.dma_start(
        out=ct_sb[:].rearrange("p (sc q) -> p sc q", q=NPC),
        in_=ct_d[:].rearrange("(sc p) q -> p sc q", p=128))

    degf_sb = sp.tile([128, 32], F32)
    degi_sb = sp.tile([128, 32], I32)
    nc.vector.tensor_tensor(out=degi_sb[:], in0=indb_sb,
                            in1=inda_sb, op=OP.subtract)
    nc.vector.tensor_scalar_add(degi_sb[:], degi_sb[:], 1)
    nc.vector.tensor_copy(out=degf_sb[:], in_=degi_sb[:])
    sq_sb = sp.tile([128, 32], F32)
    nc.scalar.activation(sq_sb[:], degf_sb[:], AF.Sqrt)
    y0_sb = sp.tile([128, 32], F32)
    nc.vector.reciprocal(y0_sb[:], sq_sb[:])
    t_sb = sp.tile([128, 32], F32)
    dinv_sb = sp.tile([128, 32], F32)
    for cur, nxt in [(y0_sb, t_sb), (t_sb, dinv_sb)]:
        tmp_sb = sp.tile([128, 32], F32, name=f"nr_{nxt.tensor.name}")
        nc.vector.tensor_tensor(out=tmp_sb[:], in0=cur[:], in1=cur[:],
                                op=OP.mult)
        nc.vector.tensor_tensor(out=tmp_sb[:], in0=tmp_sb[:],
                                in1=degf_sb[:], op=OP.mult)
        nc.vector.tensor_scalar(out=tmp_sb[:], in0=tmp_sb[:],
                                scalar1=-0.5, scalar2=1.5,
                                op0=OP.mult, op1=OP.add)
        nc.vector.tensor_tensor(out=nxt[:], in0=cur[:], in1=tmp_sb[:],
                                op=OP.mult)

    u_sb = sp.tile([128, 32], F32)
    nc.vector.tensor_tensor(out=u_sb[:], in0=x_sb, in1=dinv_sb[:],
                            op=OP.mult)
    u2_sb = sp.tile([128, 96], FP8)
    u2v = u2_sb[:].rearrange("p (c three) -> p c three", three=3)
    res_sb = sp.tile([128, 32], F32)
    for term, scale in enumerate((1.0, 64.0, 4096.0)):
        scl_sb = sp.tile([128, 32], F32, name=f"scl{term}")
        if scale == 1.0:
            src_ap = u_sb[:]
        else:
            nc.vector.tensor_scalar_mul(scl_sb[:], u_sb[:]
                                        if term == 0 else res_sb[:],
                                        scale)
            src_ap = scl_sb[:]
        nc.vector.tensor_copy(
            out=u2v[:, :, term:term + 1],
            in_=src_ap.rearrange("p (c one) -> p c one", one=1))
        if term < 2:
            back_sb = sp.tile([128, 32], F32, name=f"back{term}")
            nc.vector.tensor_copy(
                out=back_sb[:].rearrange("p (c one) -> p c one", one=1),
                in_=u2v[:, :, term:term + 1])
            if scale != 1.0:
                nc.vector.tensor_scalar_mul(back_sb[:], back_sb[:],
                                            1.0 / scale)
            nc.vector.tensor_tensor(
                out=res_sb[:], in0=(u_sb[:] if term == 0 else res_sb[:]),
                in1=back_sb[:], op=OP.subtract)

    agg_ps = [pp.tile([128, 3], F32, name=f"aps{db}") for db in range(4)]
    for db in range(4):
        for sc in range(32):
            nc.tensor.matmul(
                out=agg_ps[db][:],
                lhsT=ct_sb[:, NPC * sc + 128 * db:NPC * sc + 128 * (db + 1)],
                rhs=u2_sb[:, 3 * sc:3 * sc + 3],
                start=(sc == 0), stop=(sc == 31))
    aggt_sb = sp.tile([128, 12], F32)
    for db in range(4):
        nc.vector.tensor_copy(out=aggt_sb[:, 3 * db:3 * db + 3],
                              in_=agg_ps[db][:])
    agg_sb = sp.tile([128, 4], F32)
    av = aggt_sb[:].rearrange("p (db three) -> p db three", three=3)
    nc.vector.tensor_scalar_mul(av[:, :, 1:2], av[:, :, 1:2], 1.0 / 64)
    nc.vector.tensor_scalar_mul(av[:, :, 2:3], av[:, :, 2:3], 1.0 / 4096)
    nc.vector.tensor_reduce(out=agg_sb[:], in_=av,
                            axis=mybir.AxisListType.X, op=OP.add)

    s_sb = sp.tile([128, 4], F32)
    nc.vector.tensor_tensor(out=s_sb[:], in0=agg_sb[:],
                            in1=dinv_sb[:, 0:4], op=OP.mult)
    return s_sb, dinv_sb


def _build_kernel_general(ct_bf16=False):
    """b1 != 0 fallback: original kernel, full Wr row-parallel matvec."""
    nc = bacc.Bacc("TRN2", target_bir_lowering=False, debug=False,
                   num_devices=NCORES)

    pk_d = nc.dram_tensor("packed", [128, 96], I32, kind="ExternalInput")
    ct_dt = BF16 if ct_bf16 else FP8
    ct_d = nc.dram_tensor("ct", [N, NPC], ct_dt, kind="ExternalInput")
    wb_d = nc.dram_tensor("w1b1", [1, 2 * HID], F32, kind="ExternalInput")
    bias_d = nc.dram_tensor("bias", [1, Y], F32, kind="ExternalInput")
    wr_d = nc.dram_tensor("wr", [8 * NPC, Y], WR_DT, kind="ExternalInput")
    y_d = nc.dram_tensor("y", [1, Y], F32, kind="ExternalOutput")

    with tile.TileContext(nc) as tc:
        with (
            tc.tile_pool(name="small", bufs=1) as sp,
            tc.tile_pool(name="wr", bufs=1) as wp,
            tc.tile_pool(name="psum", bufs=1, space="PSUM") as pp,
        ):
            wbrow = sp.tile([1, 2 * HID], F32)
            nc.sync.dma_start(out=wbrow[:], in_=wb_d[:])
            w1row = wbrow[:, 0:HID]
            b1row = wbrow[:, HID:2 * HID]
            bias_sb = sp.tile([1, Y], F32)
            nc.sync.dma_start(out=bias_sb[:], in_=bias_d[:])

            s_sb, _ = _mp_subgraph(nc, sp, pp, pk_d, ct_d, ct_dt)

            ones_sb = sp.tile([1, 128], F32)
            nc.vector.memset(ones_sb[:], 1.0)
            wb_ps = pp.tile([128, 2 * HID], F32, name="ps4")
            nc.tensor.matmul(out=wb_ps[:, 0:HID], lhsT=ones_sb[:],
                             rhs=w1row, start=True, stop=True)
            nc.tensor.matmul(out=wb_ps[:, HID:2 * HID], lhsT=ones_sb[:],
                             rhs=b1row, start=True, stop=True)
            wb_sb = sp.tile([128, 2 * HID], F32)
            nc.vector.tensor_copy(out=wb_sb[:], in_=wb_ps[:])

            h_sb = sp.tile([128, 4 * HID], BF16)
            for kk in range(HID):
                nc.vector.tensor_scalar(
                    out=h_sb[:, 4 * kk:4 * kk + 4], in0=s_sb[:],
                    scalar1=wb_sb[:, kk:kk + 1],
                    scalar2=wb_sb[:, HID + kk:HID + kk + 1],
                    op0=OP.mult, op1=OP.add)
            nc.vector.tensor_scalar_max(h_sb[:], h_sb[:], 0.0)

            # alias the MP agg banks (aps0-3) and wb bank (ps4): the PSUM
            # pool dedupes tiles by name and only 8 banks exist.  Bias is
            # added during the PSUM->SBUF copy (a DVE preload would be lost:
            # only TensorE matmuls set the has_written accumulate bits).
            y_ps = [pp.tile([1, 512], F32,
                            name=(f"aps{bk}" if bk < 4 else f"ps{bk}"))
                    for bk in range(8)]
            for t in range(32):
                wr_sb = wp.tile([128, Y], WR_DT, name=f"wr{t % 12}")
                nc.sync.dma_start(out=wr_sb[:],
                                  in_=wr_d[128 * t:128 * (t + 1), :])
                kk, c = t // 4, t % 4
                hcol = h_sb[:, 4 * kk + c:4 * kk + c + 1]
                for bk in range(8):
                    nc.tensor.matmul(out=y_ps[bk][:], lhsT=hcol,
                                     rhs=wr_sb[:, 512 * bk:512 * (bk + 1)],
                                     start=(t == 0), stop=(t == 31),
                                     skip_group_check=True)

            y_sb = sp.tile([1, Y], F32)
            for bk in range(8):
                nc.vector.tensor_tensor(
                    out=y_sb[:, 512 * bk:512 * (bk + 1)],
                    in0=y_ps[bk][:],
                    in1=bias_sb[:, 512 * bk:512 * (bk + 1)], op=OP.add)
            nc.sync.dma_start(out=y_d[:], in_=y_sb[:])

    nc.compile()
    return nc


_NC_CACHE = {}


def _get_nc(kind, ct_bf16=False):
    key = (kind, ct_bf16)
    if key not in _NC_CACHE:
        build = _build_kernel_fast if kind == "fast" else _build_kernel_general
        _NC_CACHE[key] = build(ct_bf16)
    return _NC_CACHE[key]


def _graph_prep(x, edge_index):
    """Per-core packed x/indptr arrays and dense count matrices."""
    x = np.ascontiguousarray(x, dtype=np.float32).reshape(N)
    src = np.asarray(edge_index[0], dtype=np.int64)
    dst = np.asarray(edge_index[1], dtype=np.int64)

    indeg = np.bincount(dst, minlength=N)
    indptr = np.zeros(N + 1, dtype=np.int32)
    np.cumsum(indeg, out=indptr[1:])

    packs, cts, any_bf16 = [], [], False
    p = np.arange(128)[:, None]
    for k in range(NCORES):
        rot = (np.arange(32) + 4 * k) % 32          # column rotation
        g = 128 * rot[None, :] + p                  # [128, 32] global node ids

        mask = (dst >= NPC * k) & (dst < NPC * (k + 1))
        ck = np.zeros((NPC, N), dtype=np.float32)
        np.add.at(ck, (dst[mask] - NPC * k, src[mask]), 1.0)
        ck[np.arange(NPC), NPC * k + np.arange(NPC)] += 1.0
        # counts <= 8 are exact in fp8e4m3; fall back to bf16 otherwise
        any_bf16 = any_bf16 or bool(ck.max() > 8)
        srcperm = g.T.reshape(-1)                   # [(sc i)] -> global node
        cts.append((ck, srcperm))

        packed = np.concatenate([
            x[g].astype(np.float32).view(np.int32),
            indptr[g].astype(np.int32),
            indptr[g + 1].astype(np.int32)], axis=1)
        packs.append(np.ascontiguousarray(packed))
    ct_np = ml_dtypes.bfloat16 if any_bf16 else ml_dtypes.float8_e4m3
    cts = [np.ascontiguousarray(ck[:, srcperm].T).astype(ct_np)
           for ck, srcperm in cts]
    return packs, cts, any_bf16


def _host_prep_fast(x, edge_index, W1, b1, Wr, br):
    """Graph layout plus the W1->A weight fold (b1 == 0 only)."""
    packs, cts, any_bf16 = _graph_prep(x, edge_index)
    W1v = np.ascontiguousarray(W1, dtype=np.float32).reshape(HID)
    Wr3 = np.ascontiguousarray(Wr, dtype=np.float32).reshape(N, HID, Y)

    # A+/-[n, :] = sum_k max(+/-W1_k, 0) * Wr[8n+k, :]
    w1p = np.maximum(W1v, 0.0)
    w1m = np.maximum(-W1v, 0.0)
    ap = np.tensordot(Wr3, w1p, axes=([1], [0]))   # [N, Y]
    am = np.tensordot(Wr3, w1m, axes=([1], [0]))   # [N, Y]
    a_all = np.ascontiguousarray(
        np.concatenate([ap, -am], axis=0)).astype(WR_NP)  # [2N, Y]

    p = np.arange(128)[:, None]
    mask = (p // 16 == np.arange(32)[None, :] % 8).astype(np.float32)
    mask[:, 0:8] = 0.0                              # column 0 is static
    perm = (p % 16 == np.arange(128)[None, :] % 16).astype(np.float32)
    scalevec = np.tile(np.array([[1.0, 1.0 / 64, 1.0 / 4096]],
                                dtype=np.float32), (128, 1))

    t = np.arange(32)[None, :]
    in_maps = []
    for k in range(NCORES):
        # wrapbase[m, t] = own-node A-row id of gather slot (m, t)
        wrapbase = (512 * k + 128 * (t // 8) + 16 * (t % 8)
                    + p % 16).astype(np.float32)
        consts = np.concatenate([mask, perm, wrapbase, scalevec], axis=1)
        packed2 = np.concatenate([packs[k], consts.view(np.int32)], axis=1)
        # static column 0: rows = A+ then A- for nodes 512k .. 512k+127
        # (coeffs are relu(s) and relu(-s), both nonnegative)
        a0 = np.concatenate([ap[512 * k:512 * k + 128],
                             am[512 * k:512 * k + 128]], axis=0)
        in_maps.append({
            "packed": np.ascontiguousarray(packed2),
            "ct": cts[k],
            "a_all": a_all,
            "a0": np.ascontiguousarray(a0.astype(WR_NP)),
        })
    return in_maps, any_bf16


def _host_prep_general(x, edge_index, W1, b1, Wr, br):
    packs, cts, any_bf16 = _graph_prep(x, edge_index)
    W1v = np.ascontiguousarray(W1, dtype=np.float32).reshape(1, HID)
    b1v = np.ascontiguousarray(b1, dtype=np.float32).reshape(1, HID)
    brv = np.ascontiguousarray(br, dtype=np.float32).reshape(1, Y)
    Wr3 = np.ascontiguousarray(Wr, dtype=np.float32).reshape(N, HID, Y)

    in_maps = []
    for k in range(NCORES):
        wr_core = np.ascontiguousarray(
            Wr3[NPC * k:NPC * (k + 1)].transpose(1, 0, 2).reshape(8 * NPC, Y),
            dtype=np.float32).astype(WR_NP)
        in_maps.append({
            "packed": packs[k],
            "ct": cts[k],
            "w1b1": np.concatenate([W1v, b1v], axis=1),
            "bias": brv if k == 0 else np.zeros((1, Y), dtype=np.float32),
            "wr": wr_core,
        })
    return in_maps, any_bf16


def _run_fast(x, edge_index, W1, b1, Wr, br, _trace):
    in_maps, ct_bf16 = _host_prep_fast(x, edge_index, W1, b1, Wr, br)
    nc = _get_nc("fast", ct_bf16)
    try:
        res = run_bass_kernel_spmd(nc, in_maps, list(range(NCORES)),
                                   trace=_trace)
    except Exception:
        # one retry: recovers from transiently-poisoned device state
        res = run_bass_kernel_spmd(nc, in_maps, list(range(NCORES)),
                                   trace=_trace)
    y = np.asarray(br, dtype=np.float64).reshape(Y).copy()
    for k in range(NCORES):
        for i in range(4):
            yk = np.asarray(res.results[k][f"y{i}"]).astype(np.float64)
            y += yk.reshape(128, 32).T.reshape(Y)
    return y, res


def _run_general(x, edge_index, W1, b1, Wr, br, _trace):
    in_maps, ct_bf16 = _host_prep_general(x, edge_index, W1, b1, Wr, br)
    nc = _get_nc("general", ct_bf16)
    try:
        res = run_bass_kernel_spmd(nc, in_maps, list(range(NCORES)),
                                   trace=_trace)
    except Exception:
        res = run_bass_kernel_spmd(nc, in_maps, list(range(NCORES)),
                                   trace=_trace)
    y = np.zeros(Y, dtype=np.float64)
    for k in range(NCORES):
        y += np.asarray(res.results[k]["y"]).reshape(Y).astype(np.float64)
    return y, res


def kernel(x, edge_index, W1, b1, Wr, br, _trace=False):
    fast = not np.asarray(b1, dtype=np.float32).any()
    y = res = None
    if fast:
        try:
            y, res = _run_fast(x, edge_index, W1, b1, Wr, br, _trace)
        except Exception:
            y = None  # e.g. gather/gpsimd library unavailable: use fallback
    if y is None:
        y, res = _run_general(x, edge_index, W1, b1, Wr, br, _trace)
    out = y.astype(np.float32)
    if _trace:
        return out, res
    return out


# revision 26
# speedup vs baseline: 1.0164x; 1.0058x over previous
"""Trainium2 Bass kernel for GCN(1->8) + flatten + big regression matvec.

Model (reference):
    h = GCNConv(x[4096,1], edge_index[2,131072], W1[1,8], b1[8])   # [4096, 8]
    h = relu(h.reshape(-1))                                        # [32768]
    y = h @ Wr[32768, 4096] + br                                   # [4096]

Since x is [N,1] and W1 is [1,8], the GCN collapses to a per-node scalar
    s[d] = dinv[d] * sum_s C'[d, s] * u[s],   u = x * dinv,
    dinv = 1/sqrt(1 + indeg),   C' = edge-count matrix + I,
and h[d,k] = relu(s[d]*W1[k] + b1[k]).

Fast path (b1 == 0, the spec's fill): relu(s*W1_k) factorizes as
    relu(s*W1_k) = max(W1_k,0)*max(s,0) + max(-W1_k,0)*max(-s,0),
so the 32768-deep regression contraction collapses to one row per node:
    y = sum_n s_n * B[n,:] + br,
    B[n,:] = A+[n,:] if s_n >= 0 else -A-[n,:],
    A+/-[n,:] = sum_k max(+/-W1_k, 0) * Wr[8n+k, :]   (host weight fold).
Each core owns 512 nodes (grid columns 0..3 after the SPMD column
rotation): it runs the message passing for them (dense fp8 C' matmul),
then column 0 is served STATICALLY (A+/A- chunks stream in while the
sign pipeline runs, with relu(s)/relu(-s) coefficients) and columns 1..3
by sign-selected GPSIMD dma_gathers over A_all = [A+; -A-] (half the
HBM bytes of a static A+/A- load).  The A tiles are the stationary
(lhsT) matmul operand so the PE streams one output column per chunk.
Per-chunk partial sums go straight from PSUM to HBM; the host adds the
partials and br during the cross-core reduction.

Device index pipeline for the gathers:
  neg4096[p,c] = 4096*(agg[p,c] < 0)                         (DVE, f32)
  rhs[p,8c+g]  = neg4096[p,c] * mask[p,8c+g]                 (DVE x3)
  idx[m,t]     = sum_p perm[p,m]*rhs[p,t]                    (PE, f32)
  idxs         = int16(idx + wrapbase)                       (DVE)
where mask[p,t] = (p//16 == t%8), perm[p,m] = (p%16 == m%16), and
wrapbase[m,t] holds the A-row id of slot (m,t)'s node: this lands the
row-ids in dma_gather's "wrapped in 16 partitions, replicated per Q7
core" index layout (slot (q,t) holds the id for gather position 16t+q).
sign(s) == sign(agg) since dinv > 0, so the index pipeline races ahead
of s.

General path (b1 != 0): original row-parallel kernel over the full Wr.
"""

import numpy as np
import ml_dtypes

import concourse.bacc as bacc
import concourse.bass as bass
import concourse.mybir as mybir
import concourse.tile as tile
from concourse import library_config
from concourse.bass_utils import run_bass_kernel_spmd

N = 4096            # nodes
HID = 8             # GCN hidden dim
Y = 4096            # output dim
NCORES = 8
NPC = N // NCORES   # 512 nodes per core
WR_DT = mybir.dt.bfloat16
WR_NP = ml_dtypes.bfloat16

F32 = mybir.dt.float32
FP8 = mybir.dt.float8e4
BF16 = mybir.dt.bfloat16
I32 = mybir.dt.int32
I16 = mybir.dt.int16
AF = mybir.ActivationFunctionType
OP = mybir.AluOpType

CT_SPLIT = 4
# packed2 cols (int32 words): 0:32 x, 32:64 deg+1 (f32), then consts:
# 64:80 mask (bf16), 80:144 perm (bf16), 144:176 wrapbase, 176:179 scalevec
IC_COLS = 32 + 32 + 16 + 64 + 32 + 3


def _build_kernel_fast(ct_bf16=False):
    """b1 == 0 path: folded-A matvec, static col 0 + sign-gather cols 1-3."""
    nc = bacc.Bacc("TRN2", target_bir_lowering=False, debug=False,
                   num_devices=NCORES)

    pk_d = nc.dram_tensor("packed", [128, IC_COLS], I32, kind="ExternalInput")
    ct_dt = BF16 if ct_bf16 else FP8
    ct_d = nc.dram_tensor("ct", [N, NPC], ct_dt, kind="ExternalInput")
    a_d = nc.dram_tensor("a_all", [2 * N, Y], WR_DT, kind="ExternalInput")
    a0_d = nc.dram_tensor("a0", [256, Y], WR_DT, kind="ExternalInput")
    y_ds = [nc.dram_tensor(f"y{i}", [128, 32], F32, kind="ExternalOutput")
            for i in range(4)]

    with tile.TileContext(nc) as tc:
        with (
            tc.tile_pool(name="small", bufs=1) as sp,
            tc.tile_pool(name="bmat", bufs=1) as wp,
        ):
            nc.gpsimd.load_library(library_config.mlp)

            # ---- DMAs: ct (big chunk first hides the HWDGE pipeline so the
            # DMA engines never idle), then packed+consts, then static A0 ----
            pk_sb = sp.tile([128, IC_COLS], I32)
            x_sb = pk_sb[:, 0:32].bitcast(F32)
            degf_in = pk_sb[:, 32:64].bitcast(F32)
            mask = pk_sb[:, 64:80].bitcast(BF16)
            perm = pk_sb[:, 80:144].bitcast(BF16)
            wrapbase = pk_sb[:, 144:176].bitcast(F32)
            scalevec = pk_sb[:, 176:179].bitcast(F32)

            ct_sb = sp.tile([128, 32 * NPC], ct_dt)
            ctv = ct_sb[:].rearrange("p (sc q) -> p sc q", q=NPC)
            ctd = ct_d[:].rearrange("(sc p) q -> p sc q", p=128)
            # first ct chunk sized to cover the HWDGE pipeline of the packed
            # DMA behind it (zero DMA-engine gaps); packed lands early so the
            # ~2.9us DVE dinv/u chain overlaps the remaining ct chunks; the
            # tiny last chunk keeps the post-ct matmul tail short
            ct_bounds = [0, 6, 16, 26, 30, 32]
            nc.sync.dma_start(out=ctv[:, 0:6, :], in_=ctd[:, 0:6, :])
            nc.sync.dma_start(out=pk_sb[:], in_=pk_d[:])
            for h in range(1, len(ct_bounds) - 1):
                lo, hi = ct_bounds[h], ct_bounds[h + 1]
                nc.sync.dma_start(out=ctv[:, lo:hi, :], in_=ctd[:, lo:hi, :])
            a0_tiles = [wp.tile([128, Y], WR_DT, name=f"a0{i}")
                        for i in range(2)]
            for i in range(2):
                nc.sync.dma_start(out=a0_tiles[i][:],
                                  in_=a0_d[128 * i:128 * (i + 1), :])

            # ---- deg+1 (host-shipped) -> dinv (Rsqrt + two Newton steps) ----
            sq_sb = sp.tile([128, 32], F32)
            nc.scalar.activation(sq_sb[:], degf_in, AF.Sqrt)
            y0_sb = sp.tile([128, 32], F32)
            nc.vector.reciprocal(y0_sb[:], sq_sb[:])
            t_sb = sp.tile([128, 32], F32)
            dinv_sb = sp.tile([128, 32], F32)
            for cur, nxt in [(y0_sb, t_sb), (t_sb, dinv_sb)]:
                tmp_sb = sp.tile([128, 32], F32, name=f"nr_{nxt.tensor.name}")
                nc.vector.tensor_tensor(out=tmp_sb[:], in0=cur[:], in1=cur[:],
                                        op=OP.mult)
                nc.vector.tensor_tensor(out=tmp_sb[:], in0=tmp_sb[:],
                                        in1=degf_in, op=OP.mult)
                nc.vector.tensor_scalar(out=tmp_sb[:], in0=tmp_sb[:],
                                        scalar1=-0.5, scalar2=1.5,
                                        op0=OP.mult, op1=OP.add)
                nc.vector.tensor_tensor(out=nxt[:], in0=cur[:], in1=tmp_sb[:],
                                        op=OP.mult)

            # ---- u = x*dinv, split into three scaled fp8 terms ----
            u_sb = sp.tile([128, 32], F32)
            nc.vector.tensor_tensor(out=u_sb[:], in0=x_sb, in1=dinv_sb[:],
                                    op=OP.mult)
            u2_sb = sp.tile([128, 96], FP8)
            u2v = u2_sb[:].rearrange("p (c three) -> p c three", three=3)
            res_sb = sp.tile([128, 32], F32)
            for term, scale in enumerate((1.0, 64.0, 4096.0)):
                scl_sb = sp.tile([128, 32], F32, name=f"scl{term}")
                if scale == 1.0:
                    src_ap = u_sb[:]
                else:
                    nc.vector.tensor_scalar_mul(scl_sb[:], u_sb[:]
                                                if term == 0 else res_sb[:],
                                                scale)
                    src_ap = scl_sb[:]
                nc.vector.tensor_copy(
                    out=u2v[:, :, term:term + 1],
                    in_=src_ap.rearrange("p (c one) -> p c one", one=1))
                if term < 2:
                    back_sb = sp.tile([128, 32], F32, name=f"back{term}")
                    nc.vector.tensor_copy(
                        out=back_sb[:].rearrange("p (c one) -> p c one",
                                                 one=1),
                        in_=u2v[:, :, term:term + 1])
                    if scale != 1.0:
                        nc.vector.tensor_scalar_mul(back_sb[:], back_sb[:],
                                                    1.0 / scale)
                    nc.vector.tensor_tensor(
                        out=res_sb[:],
                        in0=(u_sb[:] if term == 0 else res_sb[:]),
                        in1=back_sb[:], op=OP.subtract)

            with tc.tile_pool(name="psum_mp", bufs=1, space="PSUM") as pp1:
                # ---- agg = C'.u: sc-outer so matmuls chase the ct chunks
                agg_ps = [pp1.tile([128, 3], F32, name=f"aps{db}")
                          for db in range(4)]
                for sc in range(32):
                    for db in range(4):
                        nc.tensor.matmul(
                            out=agg_ps[db][:],
                            lhsT=ct_sb[:, NPC * sc + 128 * db:
                                       NPC * sc + 128 * (db + 1)],
                            rhs=u2_sb[:, 3 * sc:3 * sc + 3],
                            start=(sc == 0), stop=(sc == 31))

                # aggt[:, 3db:3db+3] = ps_db * (1, 1/64, 1/4096)
                aggt_sb = sp.tile([128, 12], F32)
                for db in range(4):
                    nc.vector.tensor_tensor(
                        out=aggt_sb[:, 3 * db:3 * db + 3],
                        in0=agg_ps[db][:], in1=scalevec, op=OP.mult)
                agg_sb = sp.tile([128, 4], F32)
                nc.vector.tensor_reduce(
                    out=agg_sb[:],
                    in_=aggt_sb[:].rearrange("p (db three) -> p db three",
                                             three=3),
                    axis=mybir.AxisListType.X, op=OP.add)

                # ---- gather indices (sign from agg; dinv > 0) ----
                neg_sb = sp.tile([128, 4], F32)
                nc.vector.tensor_scalar(out=neg_sb[:], in0=agg_sb[:],
                                        scalar1=0.0, scalar2=float(N),
                                        op0=OP.is_lt, op1=OP.mult)
                rhs_sb = sp.tile([128, 32], BF16)
                for c in range(1, 4):
                    nc.vector.tensor_scalar(
                        out=rhs_sb[:, 8 * c:8 * (c + 1)],
                        in0=mask[:, 8 * c:8 * (c + 1)],
                        scalar1=neg_sb[:, c:c + 1], scalar2=None,
                        op0=OP.mult)
                nc.vector.memset(rhs_sb[:, 0:8], 0.0)
                idx_ps = pp1.tile([128, 32], F32, name="idxps")
                nc.tensor.matmul(out=idx_ps[:], lhsT=perm, rhs=rhs_sb[:],
                                 start=True, stop=True)
                idxs_sb = sp.tile([128, 32], I16)
                nc.vector.tensor_tensor(out=idxs_sb[:], in0=idx_ps[:],
                                        in1=wrapbase, op=OP.add)

                # s = dinv_own * agg; coeff = s (A- rows are pre-negated)
                co_sb = sp.tile([128, 4], BF16)
                nc.vector.tensor_tensor(out=co_sb[:], in0=agg_sb[:],
                                        in1=dinv_sb[:, 0:4], op=OP.mult)
                # relu coeffs for the static column 0
                spm_sb = sp.tile([128, 2], BF16)
                nc.vector.tensor_scalar_max(spm_sb[:, 0:1], co_sb[:, 0:1],
                                            0.0)
                nc.vector.tensor_scalar(out=spm_sb[:, 1:2],
                                        in0=co_sb[:, 0:1],
                                        scalar1=-1.0, scalar2=0.0,
                                        op0=OP.mult, op1=OP.max)

            # ---- gather own columns 1..3 (128 rows each) ----
            b_tiles = [wp.tile([128, Y], WR_DT, name=f"b{g}")
                       for g in range(1, 4)]
            for g in range(1, 4):
                nc.gpsimd.dma_gather(
                    b_tiles[g - 1][:].rearrange("p (one e) -> p one e",
                                                one=1),
                    a_d[:], idxs_sb[:, 8 * g:8 * (g + 1)], 128, 128, Y)

            # ---- partial sums straight to DRAM; host reduces + adds br ----
            with tc.tile_pool(name="psum_y", bufs=1, space="PSUM") as pp:
                y_ps = [pp.tile([128, 32], F32, name=f"yps{i}")
                        for i in range(4)]
                y_sbs = [sp.tile([128, 32], F32, name=f"ysb{i}")
                         for i in range(4)]
                # static column 0: A0+ with relu(s), A0- with relu(-s)
                for cc in range(32):
                    nc.tensor.matmul(
                        out=y_ps[0][:, cc:cc + 1],
                        lhsT=a0_tiles[0][:, 128 * cc:128 * (cc + 1)],
                        rhs=spm_sb[:, 0:1],
                        start=True, stop=False, skip_group_check=True)
                    nc.tensor.matmul(
                        out=y_ps[0][:, cc:cc + 1],
                        lhsT=a0_tiles[1][:, 128 * cc:128 * (cc + 1)],
                        rhs=spm_sb[:, 1:2],
                        start=False, stop=True, skip_group_check=True)
                nc.vector.tensor_copy(out=y_sbs[0][:], in_=y_ps[0][:])
                nc.sync.dma_start(out=y_ds[0][:], in_=y_sbs[0][:])
                for g in range(1, 4):
                    for cc in range(32):
                        nc.tensor.matmul(
                            out=y_ps[g][:, cc:cc + 1],
                            lhsT=b_tiles[g - 1][:, 128 * cc:128 * (cc + 1)],
                            rhs=co_sb[:, g:g + 1],
                            start=True, stop=True,
                            skip_group_check=True)
                    if g == 2:
                        nc.scalar.copy(out=y_sbs[g][:], in_=y_ps[g][:])
                    else:
                        nc.vector.tensor_copy(out=y_sbs[g][:],
                                              in_=y_ps[g][:])
                    nc.sync.dma_start(out=y_ds[g][:], in_=y_sbs[g][:])

    nc.compile()
    return nc


def _mp_subgraph(nc, sp, pp, pk_d, ct_d, ct_dt):
    """Message passing for the general path: packed x/indptr + dense C'
    -> s [128, 4] fp32.  Returns (s_sb, dinv_sb)."""
    pk_sb = sp.tile([128, 96], I32)
    nc.sync.dma_start(out=pk_sb[:], in_=pk_d[:])
    x_sb = pk_sb[:, 0:32].bitcast(F32)
    inda_sb = pk_sb[:, 32:64]
    indb_sb = pk_sb[:, 64:96]
    ct_sb = sp.tile([128, 32 * NPC], ct_dt)
    nc.sync.dma_start(
        out=ct_sb[:].rearrange("p (sc q) -> p sc q", q=NPC),
        in_=ct_d[:].rearrange("(sc p) q -> p sc q", p=128))

    degf_sb = sp.tile([128, 32], F32)
    degi_sb = sp.tile([128, 32], I32)
    nc.vector.tensor_tensor(out=degi_sb[:], in0=indb_sb,
                            in1=inda_sb, op=OP.subtract)
    nc.vector.tensor_scalar_add(degi_sb[:], degi_sb[:], 1)
    nc.vector.tensor_copy(out=degf_sb[:], in_=degi_sb[:])
    sq_sb = sp.tile([128, 32], F32)
    nc.scalar.activation(sq_sb[:], degf_sb[:], AF.Sqrt)
    y0_sb = sp.tile([128, 32], F32)
    nc.vector.reciprocal(y0_sb[:], sq_sb[:])
    t_sb = sp.tile([128, 32], F32)
    dinv_sb = sp.tile([128, 32], F32)
    for cur, nxt in [(y0_sb, t_sb), (t_sb, dinv_sb)]:
        tmp_sb = sp.tile([128, 32], F32, name=f"nr_{nxt.tensor.name}")
        nc.vector.tensor_tensor(out=tmp_sb[:], in0=cur[:], in1=cur[:],
                                op=OP.mult)
        nc.vector.tensor_tensor(out=tmp_sb[:], in0=tmp_sb[:],
                                in1=degf_sb[:], op=OP.mult)
        nc.vector.tensor_scalar(out=tmp_sb[:], in0=tmp_sb[:],
                                scalar1=-0.5, scalar2=1.5,
                                op0=OP.mult, op1=OP.add)
        nc.vector.tensor_tensor(out=nxt[:], in0=cur[:], in1=tmp_sb[:],
                                op=OP.mult)

    u_sb = sp.tile([128, 32], F32)
    nc.vector.tensor_tensor(out=u_sb[:], in0=x_sb, in1=dinv_sb[:],
                            op=OP.mult)
    u2_sb = sp.tile([128, 96], FP8)
    u2v = u2_sb[:].rearrange("p (c three) -> p c three", three=3)
    res_sb = sp.tile([128, 32], F32)
    for term, scale in enumerate((1.0, 64.0, 4096.0)):
        scl_sb = sp.tile([128, 32], F32, name=f"scl{term}")
        if scale == 1.0:
            src_ap = u_sb[:]
        else:
            nc.vector.tensor_scalar_mul(scl_sb[:], u_sb[:]
                                        if term == 0 else res_sb[:],
                                        scale)
            src_ap = scl_sb[:]
        nc.vector.tensor_copy(
            out=u2v[:, :, term:term + 1],
            in_=src_ap.rearrange("p (c one) -> p c one", one=1))
        if term < 2:
            back_sb = sp.tile([128, 32], F32, name=f"back{term}")
            nc.vector.tensor_copy(
                out=back_sb[:].rearrange("p (c one) -> p c one", one=1),
                in_=u2v[:, :, term:term + 1])
            if scale != 1.0:
                nc.vector.tensor_scalar_mul(back_sb[:], back_sb[:],
                                            1.0 / scale)
            nc.vector.tensor_tensor(
                out=res_sb[:], in0=(u_sb[:] if term == 0 else res_sb[:]),
                in1=back_sb[:], op=OP.subtract)

    agg_ps = [pp.tile([128, 3], F32, name=f"aps{db}") for db in range(4)]
    for db in range(4):
        for sc in range(32):
            nc.tensor.matmul(
                out=agg_ps[db][:],
                lhsT=ct_sb[:, NPC * sc + 128 * db:NPC * sc + 128 * (db + 1)],
                rhs=u2_sb[:, 3 * sc:3 * sc + 3],
                start=(sc == 0), stop=(sc == 31))
    aggt_sb = sp.tile([128, 12], F32)
    for db in range(4):
        nc.vector.tensor_copy(out=aggt_sb[:, 3 * db:3 * db + 3],
                              in_=agg_ps[db][:])
    agg_sb = sp.tile([128, 4], F32)
    av = aggt_sb[:].rearrange("p (db three) -> p db three", three=3)
    nc.vector.tensor_scalar_mul(av[:, :, 1:2], av[:, :, 1:2], 1.0 / 64)
    nc.vector.tensor_scalar_mul(av[:, :, 2:3], av[:, :, 2:3], 1.0 / 4096)
    nc.vector.tensor_reduce(out=agg_sb[:], in_=av,
                            axis=mybir.AxisListType.X, op=OP.add)

    s_sb = sp.tile([128, 4], F32)
    nc.vector.tensor_tensor(out=s_sb[:], in0=agg_sb[:],
                            in1=dinv_sb[:, 0:4], op=OP.mult)
    return s_sb, dinv_sb


def _build_kernel_general(ct_bf16=False):
    """b1 != 0 fallback: original kernel, full Wr row-parallel matvec."""
    nc = bacc.Bacc("TRN2", target_bir_lowering=False, debug=False,
                   num_devices=NCORES)

    pk_d = nc.dram_tensor("packed", [128, 96], I32, kind="ExternalInput")
    ct_dt = BF16 if ct_bf16 else FP8
    ct_d = nc.dram_tensor("ct", [N, NPC], ct_dt, kind="ExternalInput")
    wb_d = nc.dram_tensor("w1b1", [1, 2 * HID], F32, kind="ExternalInput")
    bias_d = nc.dram_tensor("bias", [1, Y], F32, kind="ExternalInput")
    wr_d = nc.dram_tensor("wr", [8 * NPC, Y], WR_DT, kind="ExternalInput")
    y_d = nc.dram_tensor("y", [1, Y], F32, kind="ExternalOutput")

    with tile.TileContext(nc) as tc:
        with (
            tc.tile_pool(name="small", bufs=1) as sp,
            tc.tile_pool(name="wr", bufs=1) as wp,
            tc.tile_pool(name="psum", bufs=1, space="PSUM") as pp,
        ):
            wbrow = sp.tile([1, 2 * HID], F32)
            nc.sync.dma_start(out=wbrow[:], in_=wb_d[:])
            w1row = wbrow[:, 0:HID]
            b1row = wbrow[:, HID:2 * HID]
            bias_sb = sp.tile([1, Y], F32)
            nc.sync.dma_start(out=bias_sb[:], in_=bias_d[:])

            s_sb, _ = _mp_subgraph(nc, sp, pp, pk_d, ct_d, ct_dt)

            ones_sb = sp.tile([1, 128], F32)
            nc.vector.memset(ones_sb[:], 1.0)
            wb_ps = pp.tile([128, 2 * HID], F32, name="ps4")
            nc.tensor.matmul(out=wb_ps[:, 0:HID], lhsT=ones_sb[:],
                             rhs=w1row, start=True, stop=True)
            nc.tensor.matmul(out=wb_ps[:, HID:2 * HID], lhsT=ones_sb[:],
                             rhs=b1row, start=True, stop=True)
            wb_sb = sp.tile([128, 2 * HID], F32)
            nc.vector.tensor_copy(out=wb_sb[:], in_=wb_ps[:])

            h_sb = sp.tile([128, 4 * HID], BF16)
            for kk in range(HID):
                nc.vector.tensor_scalar(
                    out=h_sb[:, 4 * kk:4 * kk + 4], in0=s_sb[:],
                    scalar1=wb_sb[:, kk:kk + 1],
                    scalar2=wb_sb[:, HID + kk:HID + kk + 1],
                    op0=OP.mult, op1=OP.add)
            nc.vector.tensor_scalar_max(h_sb[:], h_sb[:], 0.0)

            # alias the MP agg banks (aps0-3) and wb bank (ps4): the PSUM
            # pool dedupes tiles by name and only 8 banks exist.  Bias is
            # added during the PSUM->SBUF copy (a DVE preload would be lost:
            # only TensorE matmuls set the has_written accumulate bits).
            y_ps = [pp.tile([1, 512], F32,
                            name=(f"aps{bk}" if bk < 4 else f"ps{bk}"))
                    for bk in range(8)]
            for t in range(32):
                wr_sb = wp.tile([128, Y], WR_DT, name=f"wr{t % 12}")
                nc.sync.dma_start(out=wr_sb[:],
                                  in_=wr_d[128 * t:128 * (t + 1), :])
                kk, c = t // 4, t % 4
                hcol = h_sb[:, 4 * kk + c:4 * kk + c + 1]
                for bk in range(8):
                    nc.tensor.matmul(out=y_ps[bk][:], lhsT=hcol,
                                     rhs=wr_sb[:, 512 * bk:512 * (bk + 1)],
                                     start=(t == 0), stop=(t == 31),
                                     skip_group_check=True)

            y_sb = sp.tile([1, Y], F32)
            for bk in range(8):
                nc.vector.tensor_tensor(
                    out=y_sb[:, 512 * bk:512 * (bk + 1)],
                    in0=y_ps[bk][:],
                    in1=bias_sb[:, 512 * bk:512 * (bk + 1)], op=OP.add)
            nc.sync.dma_start(out=y_d[:], in_=y_sb[:])

    nc.compile()
    return nc


_NC_CACHE = {}


def _get_nc(kind, ct_bf16=False):
    key = (kind, ct_bf16)
    if key not in _NC_CACHE:
        build = _build_kernel_fast if kind == "fast" else _build_kernel_general
        _NC_CACHE[key] = build(ct_bf16)
    return _NC_CACHE[key]


def _graph_prep(x, edge_index):
    """Per-core packed x/indptr arrays and dense count matrices."""
    x = np.ascontiguousarray(x, dtype=np.float32).reshape(N)
    src = np.asarray(edge_index[0], dtype=np.int64)
    dst = np.asarray(edge_index[1], dtype=np.int64)

    indeg = np.bincount(dst, minlength=N)
    indptr = np.zeros(N + 1, dtype=np.int32)
    np.cumsum(indeg, out=indptr[1:])

    packs, cts, any_bf16 = [], [], False
    p = np.arange(128)[:, None]
    for k in range(NCORES):
        rot = (np.arange(32) + 4 * k) % 32          # column rotation
        g = 128 * rot[None, :] + p                  # [128, 32] global node ids

        mask = (dst >= NPC * k) & (dst < NPC * (k + 1))
        ck = np.zeros((NPC, N), dtype=np.float32)
        np.add.at(ck, (dst[mask] - NPC * k, src[mask]), 1.0)
        ck[np.arange(NPC), NPC * k + np.arange(NPC)] += 1.0
        # counts <= 8 are exact in fp8e4m3; fall back to bf16 otherwise
        any_bf16 = any_bf16 or bool(ck.max() > 8)
        srcperm = g.T.reshape(-1)                   # [(sc i)] -> global node
        cts.append((ck, srcperm))

        packed = np.concatenate([
            x[g].astype(np.float32).view(np.int32),
            indptr[g].astype(np.int32),
            indptr[g + 1].astype(np.int32)], axis=1)
        packs.append(np.ascontiguousarray(packed))
    ct_np = ml_dtypes.bfloat16 if any_bf16 else ml_dtypes.float8_e4m3
    cts = [np.ascontiguousarray(ck[:, srcperm].T).astype(ct_np)
           for ck, srcperm in cts]
    return packs, cts, any_bf16


def _host_prep_fast(x, edge_index, W1, b1, Wr, br):
    """Graph layout plus the W1->A weight fold (b1 == 0 only)."""
    packs, cts, any_bf16 = _graph_prep(x, edge_index)
    # fast path ships x and deg+1 (f32) instead of x + indptr pairs
    indeg = np.bincount(np.asarray(edge_index[1], dtype=np.int64),
                        minlength=N)
    degp1 = (indeg + 1).astype(np.float32)
    xf = np.ascontiguousarray(x, dtype=np.float32).reshape(N)
    W1v = np.ascontiguousarray(W1, dtype=np.float32).reshape(HID)
    Wr3 = np.ascontiguousarray(Wr, dtype=np.float32).reshape(N, HID, Y)

    # A+/-[n, :] = sum_k max(+/-W1_k, 0) * Wr[8n+k, :]
    w1p = np.maximum(W1v, 0.0)
    w1m = np.maximum(-W1v, 0.0)
    ap = np.tensordot(Wr3, w1p, axes=([1], [0]))   # [N, Y]
    am = np.tensordot(Wr3, w1m, axes=([1], [0]))   # [N, Y]
    a_all = np.ascontiguousarray(
        np.concatenate([ap, -am], axis=0)).astype(WR_NP)  # [2N, Y]

    p = np.arange(128)[:, None]
    mask = (p // 16 == np.arange(32)[None, :] % 8).astype(np.float32)
    mask[:, 0:8] = 0.0                              # column 0 is static
    perm = (p % 16 == np.arange(128)[None, :] % 16).astype(np.float32)
    # mask/perm hold exact small values: ship as bf16 (f32-word packed)
    maskv = np.ascontiguousarray(
        mask.astype(ml_dtypes.bfloat16)).view(np.float32)
    permv = np.ascontiguousarray(
        perm.astype(ml_dtypes.bfloat16)).view(np.float32)
    scalevec = np.tile(np.array([[1.0, 1.0 / 64, 1.0 / 4096]],
                                dtype=np.float32), (128, 1))

    t = np.arange(32)[None, :]
    in_maps = []
    for k in range(NCORES):
        rot = (np.arange(32) + 4 * k) % 32          # column rotation
        g = 128 * rot[None, :] + p                  # [128, 32] global node ids
        # wrapbase[m, t] = own-node A-row id of gather slot (m, t)
        wrapbase = (512 * k + 128 * (t // 8) + 16 * (t % 8)
                    + p % 16).astype(np.float32)
        consts = np.concatenate([xf[g], degp1[g], maskv, permv, wrapbase,
                                 scalevec], axis=1)
        packed2 = np.ascontiguousarray(consts).view(np.int32)
        # static column 0: rows = A+ then A- for nodes 512k .. 512k+127
        # (coeffs are relu(s) and relu(-s), both nonnegative)
        a0 = np.concatenate([ap[512 * k:512 * k + 128],
                             am[512 * k:512 * k + 128]], axis=0)
        in_maps.append({
            "packed": np.ascontiguousarray(packed2),
            "ct": cts[k],
            "a_all": a_all,
            "a0": np.ascontiguousarray(a0.astype(WR_NP)),
        })
    return in_maps, any_bf16


def _host_prep_general(x, edge_index, W1, b1, Wr, br):
    packs, cts, any_bf16 = _graph_prep(x, edge_index)
    W1v = np.ascontiguousarray(W1, dtype=np.float32).reshape(1, HID)
    b1v = np.ascontiguousarray(b1, dtype=np.float32).reshape(1, HID)
    brv = np.ascontiguousarray(br, dtype=np.float32).reshape(1, Y)
    Wr3 = np.ascontiguousarray(Wr, dtype=np.float32).reshape(N, HID, Y)

    in_maps = []
    for k in range(NCORES):
        wr_core = np.ascontiguousarray(
            Wr3[NPC * k:NPC * (k + 1)].transpose(1, 0, 2).reshape(8 * NPC, Y),
            dtype=np.float32).astype(WR_NP)
        in_maps.append({
            "packed": packs[k],
            "ct": cts[k],
            "w1b1": np.concatenate([W1v, b1v], axis=1),
            "bias": brv if k == 0 else np.zeros((1, Y), dtype=np.float32),
            "wr": wr_core,
        })
    return in_maps, any_bf16


def _run_fast(x, edge_index, W1, b1, Wr, br, _trace):
    in_maps, ct_bf16 = _host_prep_fast(x, edge_index, W1, b1, Wr, br)
    nc = _get_nc("fast", ct_bf16)
    try:
        res = run_bass_kernel_spmd(nc, in_maps, list(range(NCORES)),
                                   trace=_trace)
    except Exception:
        # one retry: recovers from transiently-poisoned device state
        res = run_bass_kernel_spmd(nc, in_maps, list(range(NCORES)),
                                   trace=_trace)
    y = np.asarray(br, dtype=np.float64).reshape(Y).copy()
    for k in range(NCORES):
        for i in range(4):
            yk = np.asarray(res.results[k][f"y{i}"]).astype(np.float64)
            y += yk.reshape(128, 32).T.reshape(Y)
    return y, res


def _run_general(x, edge_index, W1, b1, Wr, br, _trace):
    in_maps, ct_bf16 = _host_prep_general(x, edge_index, W1, b1, Wr, br)
    nc = _get_nc("general", ct_bf16)
    try:
        res = run_bass_kernel_spmd(nc, in_maps, list(range(NCORES)),
                                   trace=_trace)
    except Exception:
        res = run_bass_kernel_spmd(nc, in_maps, list(range(NCORES)),
                                   trace=_trace)
    y = np.zeros(Y, dtype=np.float64)
    for k in range(NCORES):
        y += np.asarray(res.results[k]["y"]).reshape(Y).astype(np.float64)
    return y, res


def kernel(x, edge_index, W1, b1, Wr, br, _trace=False):
    fast = not np.asarray(b1, dtype=np.float32).any()
    y = res = None
    if fast:
        try:
            y, res = _run_fast(x, edge_index, W1, b1, Wr, br, _trace)
        except Exception:
            y = None  # e.g. gather/gpsimd library unavailable: use fallback
    if y is None:
        y, res = _run_general(x, edge_index, W1, b1, Wr, br, _trace)
    out = y.astype(np.float32)
    if _trace:
        return out, res
    return out


# revision 28
# speedup vs baseline: 1.0177x; 1.0013x over previous
"""Trainium2 Bass kernel for GCN(1->8) + flatten + big regression matvec.

Model (reference):
    h = GCNConv(x[4096,1], edge_index[2,131072], W1[1,8], b1[8])   # [4096, 8]
    h = relu(h.reshape(-1))                                        # [32768]
    y = h @ Wr[32768, 4096] + br                                   # [4096]

Since x is [N,1] and W1 is [1,8], the GCN collapses to a per-node scalar
    s[d] = dinv[d] * sum_s C'[d, s] * u[s],   u = x * dinv,
    dinv = 1/sqrt(1 + indeg),   C' = edge-count matrix + I,
and h[d,k] = relu(s[d]*W1[k] + b1[k]).

Fast path (b1 == 0, the spec's fill): relu(s*W1_k) factorizes as
    relu(s*W1_k) = max(W1_k,0)*max(s,0) + max(-W1_k,0)*max(-s,0),
so the 32768-deep regression contraction collapses to one row per node:
    y = sum_n s_n * B[n,:] + br,
    B[n,:] = A+[n,:] if s_n >= 0 else -A-[n,:],
    A+/-[n,:] = sum_k max(+/-W1_k, 0) * Wr[8n+k, :]   (host weight fold).
Each core owns 512 nodes (grid columns 0..3 after the SPMD column
rotation): it runs the message passing for them (dense fp8 C' matmul),
then column 0 is served STATICALLY (A+/A- chunks stream in while the
sign pipeline runs, with relu(s)/relu(-s) coefficients) and columns 1..3
by sign-selected GPSIMD dma_gathers over A_all = [A+; -A-] (half the
HBM bytes of a static A+/A- load).  The A tiles are the stationary
(lhsT) matmul operand so the PE streams one output column per chunk.
Per-chunk partial sums go straight from PSUM to HBM; the host adds the
partials and br during the cross-core reduction.

Device index pipeline for the gathers:
  neg4096[p,c] = 4096*(agg[p,c] < 0)                         (DVE, f32)
  rhs[p,8c+g]  = neg4096[p,c] * mask[p,8c+g]                 (DVE x3)
  idx[m,t]     = sum_p perm[p,m]*rhs[p,t]                    (PE, f32)
  idxs         = int16(idx + wrapbase)                       (DVE)
where mask[p,t] = (p//16 == t%8), perm[p,m] = (p%16 == m%16), and
wrapbase[m,t] holds the A-row id of slot (m,t)'s node: this lands the
row-ids in dma_gather's "wrapped in 16 partitions, replicated per Q7
core" index layout (slot (q,t) holds the id for gather position 16t+q).
sign(s) == sign(agg) since dinv > 0, so the index pipeline races ahead
of s.

General path (b1 != 0): original row-parallel kernel over the full Wr.
"""

import numpy as np
import ml_dtypes

import concourse.bacc as bacc
import concourse.bass as bass
import concourse.mybir as mybir
import concourse.tile as tile
from concourse import library_config
from concourse.bass_utils import run_bass_kernel_spmd

N = 4096            # nodes
HID = 8             # GCN hidden dim
Y = 4096            # output dim
NCORES = 8
NPC = N // NCORES   # 512 nodes per core
WR_DT = mybir.dt.bfloat16
WR_NP = ml_dtypes.bfloat16

F32 = mybir.dt.float32
FP8 = mybir.dt.float8e4
BF16 = mybir.dt.bfloat16
I32 = mybir.dt.int32
I16 = mybir.dt.int16
AF = mybir.ActivationFunctionType
OP = mybir.AluOpType

CT_SPLIT = 4
# packed2 cols (int32 words): 0:32 x, 32:64 deg+1 (f32), then consts:
# 64:80 mask (bf16), 80:144 perm (bf16), 144:176 wrapbase, 176:179 scalevec
IC_COLS = 32 + 32 + 16 + 64 + 32 + 3


def _build_kernel_fast(ct_bf16=False):
    """b1 == 0 path: folded-A matvec, static col 0 + sign-gather cols 1-3."""
    nc = bacc.Bacc("TRN2", target_bir_lowering=False, debug=False,
                   num_devices=NCORES)

    pk_d = nc.dram_tensor("packed", [128, IC_COLS], I32, kind="ExternalInput")
    ct_dt = BF16 if ct_bf16 else FP8
    ct_d = nc.dram_tensor("ct", [N, NPC], ct_dt, kind="ExternalInput")
    a_d = nc.dram_tensor("a_all", [2 * N, Y], WR_DT, kind="ExternalInput")
    a0_d = nc.dram_tensor("a0", [256, Y], WR_DT, kind="ExternalInput")
    y_ds = [nc.dram_tensor(f"y{i}", [128, 32], BF16, kind="ExternalOutput")
            for i in range(4)]

    with tile.TileContext(nc) as tc:
        with (
            tc.tile_pool(name="small", bufs=1) as sp,
            tc.tile_pool(name="bmat", bufs=1) as wp,
        ):
            nc.gpsimd.load_library(library_config.mlp)

            # ---- DMAs: ct (big chunk first hides the HWDGE pipeline so the
            # DMA engines never idle), then packed+consts, then static A0 ----
            pk_sb = sp.tile([128, IC_COLS], I32)
            x_sb = pk_sb[:, 0:32].bitcast(F32)
            degf_in = pk_sb[:, 32:64].bitcast(F32)
            mask = pk_sb[:, 64:80].bitcast(BF16)
            perm = pk_sb[:, 80:144].bitcast(BF16)
            wrapbase = pk_sb[:, 144:176].bitcast(F32)
            scalevec = pk_sb[:, 176:179].bitcast(F32)

            ct_sb = sp.tile([128, 32 * NPC], ct_dt)
            ctv = ct_sb[:].rearrange("p (sc q) -> p sc q", q=NPC)
            ctd = ct_d[:].rearrange("(sc p) q -> p sc q", p=128)
            # first ct chunk sized to cover the HWDGE pipeline of the packed
            # DMA behind it (zero DMA-engine gaps); packed lands early so the
            # ~2.9us DVE dinv/u chain overlaps the remaining ct chunks; the
            # tiny last chunk keeps the post-ct matmul tail short
            ct_bounds = [0, 6, 16, 26, 30, 32]
            nc.sync.dma_start(out=ctv[:, 0:6, :], in_=ctd[:, 0:6, :])
            nc.sync.dma_start(out=pk_sb[:], in_=pk_d[:])
            for h in range(1, len(ct_bounds) - 1):
                lo, hi = ct_bounds[h], ct_bounds[h + 1]
                nc.sync.dma_start(out=ctv[:, lo:hi, :], in_=ctd[:, lo:hi, :])
            a0_tiles = [wp.tile([128, Y], WR_DT, name=f"a0{i}")
                        for i in range(2)]
            for i in range(2):
                nc.sync.dma_start(out=a0_tiles[i][:],
                                  in_=a0_d[128 * i:128 * (i + 1), :])

            # ---- deg+1 (host-shipped) -> dinv (Rsqrt + two Newton steps) ----
            sq_sb = sp.tile([128, 32], F32)
            nc.scalar.activation(sq_sb[:], degf_in, AF.Sqrt)
            y0_sb = sp.tile([128, 32], F32)
            nc.vector.reciprocal(y0_sb[:], sq_sb[:])
            t_sb = sp.tile([128, 32], F32)
            dinv_sb = sp.tile([128, 32], F32)
            for cur, nxt in [(y0_sb, t_sb), (t_sb, dinv_sb)]:
                tmp_sb = sp.tile([128, 32], F32, name=f"nr_{nxt.tensor.name}")
                nc.vector.tensor_tensor(out=tmp_sb[:], in0=cur[:], in1=cur[:],
                                        op=OP.mult)
                nc.vector.tensor_tensor(out=tmp_sb[:], in0=tmp_sb[:],
                                        in1=degf_in, op=OP.mult)
                nc.vector.tensor_scalar(out=tmp_sb[:], in0=tmp_sb[:],
                                        scalar1=-0.5, scalar2=1.5,
                                        op0=OP.mult, op1=OP.add)
                nc.vector.tensor_tensor(out=nxt[:], in0=cur[:], in1=tmp_sb[:],
                                        op=OP.mult)

            # ---- u = x*dinv, split into three scaled fp8 terms ----
            u_sb = sp.tile([128, 32], F32)
            nc.vector.tensor_tensor(out=u_sb[:], in0=x_sb, in1=dinv_sb[:],
                                    op=OP.mult)
            u2_sb = sp.tile([128, 96], FP8)
            u2v = u2_sb[:].rearrange("p (c three) -> p c three", three=3)
            res_sb = sp.tile([128, 32], F32)
            for term, scale in enumerate((1.0, 64.0, 4096.0)):
                scl_sb = sp.tile([128, 32], F32, name=f"scl{term}")
                if scale == 1.0:
                    src_ap = u_sb[:]
                else:
                    nc.vector.tensor_scalar_mul(scl_sb[:], u_sb[:]
                                                if term == 0 else res_sb[:],
                                                scale)
                    src_ap = scl_sb[:]
                nc.vector.tensor_copy(
                    out=u2v[:, :, term:term + 1],
                    in_=src_ap.rearrange("p (c one) -> p c one", one=1))
                if term < 2:
                    back_sb = sp.tile([128, 32], F32, name=f"back{term}")
                    nc.vector.tensor_copy(
                        out=back_sb[:].rearrange("p (c one) -> p c one",
                                                 one=1),
                        in_=u2v[:, :, term:term + 1])
                    if scale != 1.0:
                        nc.vector.tensor_scalar_mul(back_sb[:], back_sb[:],
                                                    1.0 / scale)
                    nc.vector.tensor_tensor(
                        out=res_sb[:],
                        in0=(u_sb[:] if term == 0 else res_sb[:]),
                        in1=back_sb[:], op=OP.subtract)

            with tc.tile_pool(name="psum_mp", bufs=1, space="PSUM") as pp1:
                # ---- agg = C'.u: sc-outer so matmuls chase the ct chunks
                agg_ps = [pp1.tile([128, 3], F32, name=f"aps{db}")
                          for db in range(4)]
                for sc in range(32):
                    for db in range(4):
                        nc.tensor.matmul(
                            out=agg_ps[db][:],
                            lhsT=ct_sb[:, NPC * sc + 128 * db:
                                       NPC * sc + 128 * (db + 1)],
                            rhs=u2_sb[:, 3 * sc:3 * sc + 3],
                            start=(sc == 0), stop=(sc == 31))

                # aggt[:, 3db:3db+3] = ps_db * (1, 1/64, 1/4096)
                aggt_sb = sp.tile([128, 12], F32)
                for db in range(4):
                    nc.vector.tensor_tensor(
                        out=aggt_sb[:, 3 * db:3 * db + 3],
                        in0=agg_ps[db][:], in1=scalevec, op=OP.mult)
                agg_sb = sp.tile([128, 4], F32)
                nc.vector.tensor_reduce(
                    out=agg_sb[:],
                    in_=aggt_sb[:].rearrange("p (db three) -> p db three",
                                             three=3),
                    axis=mybir.AxisListType.X, op=OP.add)

                # ---- gather indices (sign from agg; dinv > 0) ----
                neg_sb = sp.tile([128, 4], F32)
                nc.vector.tensor_scalar(out=neg_sb[:], in0=agg_sb[:],
                                        scalar1=0.0, scalar2=float(N),
                                        op0=OP.is_lt, op1=OP.mult)
                rhs_sb = sp.tile([128, 32], BF16)
                for c in range(1, 4):
                    nc.vector.tensor_scalar(
                        out=rhs_sb[:, 8 * c:8 * (c + 1)],
                        in0=mask[:, 8 * c:8 * (c + 1)],
                        scalar1=neg_sb[:, c:c + 1], scalar2=None,
                        op0=OP.mult)
                nc.vector.memset(rhs_sb[:, 0:8], 0.0)
                idx_ps = pp1.tile([128, 32], F32, name="idxps")
                nc.tensor.matmul(out=idx_ps[:], lhsT=perm, rhs=rhs_sb[:],
                                 start=True, stop=True)
                idxs_sb = sp.tile([128, 32], I16)
                nc.vector.tensor_tensor(out=idxs_sb[:], in0=idx_ps[:],
                                        in1=wrapbase, op=OP.add)

                # s = dinv_own * agg; coeff = s (A- rows are pre-negated)
                co_sb = sp.tile([128, 4], BF16)
                nc.vector.tensor_tensor(out=co_sb[:], in0=agg_sb[:],
                                        in1=dinv_sb[:, 0:4], op=OP.mult)
                # relu coeffs for the static column 0
                spm_sb = sp.tile([128, 2], BF16)
                nc.vector.tensor_scalar_max(spm_sb[:, 0:1], co_sb[:, 0:1],
                                            0.0)
                nc.vector.tensor_scalar(out=spm_sb[:, 1:2],
                                        in0=co_sb[:, 0:1],
                                        scalar1=-1.0, scalar2=0.0,
                                        op0=OP.mult, op1=OP.max)

            # ---- gather own columns 1..3 (128 rows each) ----
            b_tiles = [wp.tile([128, Y], WR_DT, name=f"b{g}")
                       for g in range(1, 4)]
            for g in range(1, 4):
                nc.gpsimd.dma_gather(
                    b_tiles[g - 1][:].rearrange("p (one e) -> p one e",
                                                one=1),
                    a_d[:], idxs_sb[:, 8 * g:8 * (g + 1)], 128, 128, Y)

            # ---- partial sums straight to DRAM; host reduces + adds br ----
            with tc.tile_pool(name="psum_y", bufs=1, space="PSUM") as pp:
                y_ps = [pp.tile([128, 32], F32, name=f"yps{i}")
                        for i in range(4)]
                y_sbs = [sp.tile([128, 32], BF16, name=f"ysb{i}")
                         for i in range(4)]
                # static column 0: A0+ with relu(s), A0- with relu(-s)
                for cc in range(32):
                    nc.tensor.matmul(
                        out=y_ps[0][:, cc:cc + 1],
                        lhsT=a0_tiles[0][:, 128 * cc:128 * (cc + 1)],
                        rhs=spm_sb[:, 0:1],
                        start=True, stop=False, skip_group_check=True)
                    nc.tensor.matmul(
                        out=y_ps[0][:, cc:cc + 1],
                        lhsT=a0_tiles[1][:, 128 * cc:128 * (cc + 1)],
                        rhs=spm_sb[:, 1:2],
                        start=False, stop=True, skip_group_check=True)
                nc.vector.tensor_copy(out=y_sbs[0][:], in_=y_ps[0][:])
                nc.sync.dma_start(out=y_ds[0][:], in_=y_sbs[0][:])
                for g in range(1, 4):
                    for cc in range(32):
                        nc.tensor.matmul(
                            out=y_ps[g][:, cc:cc + 1],
                            lhsT=b_tiles[g - 1][:, 128 * cc:128 * (cc + 1)],
                            rhs=co_sb[:, g:g + 1],
                            start=True, stop=True,
                            skip_group_check=True)
                    if g == 2:
                        nc.scalar.copy(out=y_sbs[g][:], in_=y_ps[g][:])
                    else:
                        nc.vector.tensor_copy(out=y_sbs[g][:],
                                              in_=y_ps[g][:])
                    nc.sync.dma_start(out=y_ds[g][:], in_=y_sbs[g][:])

    nc.compile()
    return nc


def _mp_subgraph(nc, sp, pp, pk_d, ct_d, ct_dt):
    """Message passing for the general path: packed x/indptr + dense C'
    -> s [128, 4] fp32.  Returns (s_sb, dinv_sb)."""
    pk_sb = sp.tile([128, 96], I32)
    nc.sync.dma_start(out=pk_sb[:], in_=pk_d[:])
    x_sb = pk_sb[:, 0:32].bitcast(F32)
    inda_sb = pk_sb[:, 32:64]
    indb_sb = pk_sb[:, 64:96]
    ct_sb = sp.tile([128, 32 * NPC], ct_dt)
    nc.sync.dma_start(
        out=ct_sb[:].rearrange("p (sc q) -> p sc q", q=NPC),
        in_=ct_d[:].rearrange("(sc p) q -> p sc q", p=128))

    degf_sb = sp.tile([128, 32], F32)
    degi_sb = sp.tile([128, 32], I32)
    nc.vector.tensor_tensor(out=degi_sb[:], in0=indb_sb,
                            in1=inda_sb, op=OP.subtract)
    nc.vector.tensor_scalar_add(degi_sb[:], degi_sb[:], 1)
    nc.vector.tensor_copy(out=degf_sb[:], in_=degi_sb[:])
    sq_sb = sp.tile([128, 32], F32)
    nc.scalar.activation(sq_sb[:], degf_sb[:], AF.Sqrt)
    y0_sb = sp.tile([128, 32], F32)
    nc.vector.reciprocal(y0_sb[:], sq_sb[:])
    t_sb = sp.tile([128, 32], F32)
    dinv_sb = sp.tile([128, 32], F32)
    for cur, nxt in [(y0_sb, t_sb), (t_sb, dinv_sb)]:
        tmp_sb = sp.tile([128, 32], F32, name=f"nr_{nxt.tensor.name}")
        nc.vector.tensor_tensor(out=tmp_sb[:], in0=cur[:], in1=cur[:],
                                op=OP.mult)
        nc.vector.tensor_tensor(out=tmp_sb[:], in0=tmp_sb[:],
                                in1=degf_sb[:], op=OP.mult)
        nc.vector.tensor_scalar(out=tmp_sb[:], in0=tmp_sb[:],
                                scalar1=-0.5, scalar2=1.5,
                                op0=OP.mult, op1=OP.add)
        nc.vector.tensor_tensor(out=nxt[:], in0=cur[:], in1=tmp_sb[:],
                                op=OP.mult)

    u_sb = sp.tile([128, 32], F32)
    nc.vector.tensor_tensor(out=u_sb[:], in0=x_sb, in1=dinv_sb[:],
                            op=OP.mult)
    u2_sb = sp.tile([128, 96], FP8)
    u2v = u2_sb[:].rearrange("p (c three) -> p c three", three=3)
    res_sb = sp.tile([128, 32], F32)
    for term, scale in enumerate((1.0, 64.0, 4096.0)):
        scl_sb = sp.tile([128, 32], F32, name=f"scl{term}")
        if scale == 1.0:
            src_ap = u_sb[:]
        else:
            nc.vector.tensor_scalar_mul(scl_sb[:], u_sb[:]
                                        if term == 0 else res_sb[:],
                                        scale)
            src_ap = scl_sb[:]
        nc.vector.tensor_copy(
            out=u2v[:, :, term:term + 1],
            in_=src_ap.rearrange("p (c one) -> p c one", one=1))
        if term < 2:
            back_sb = sp.tile([128, 32], F32, name=f"back{term}")
            nc.vector.tensor_copy(
                out=back_sb[:].rearrange("p (c one) -> p c one", one=1),
                in_=u2v[:, :, term:term + 1])
            if scale != 1.0:
                nc.vector.tensor_scalar_mul(back_sb[:], back_sb[:],
                                            1.0 / scale)
            nc.vector.tensor_tensor(
                out=res_sb[:], in0=(u_sb[:] if term == 0 else res_sb[:]),
                in1=back_sb[:], op=OP.subtract)

    agg_ps = [pp.tile([128, 3], F32, name=f"aps{db}") for db in range(4)]
    for db in range(4):
        for sc in range(32):
            nc.tensor.matmul(
                out=agg_ps[db][:],
                lhsT=ct_sb[:, NPC * sc + 128 * db:NPC * sc + 128 * (db + 1)],
                rhs=u2_sb[:, 3 * sc:3 * sc + 3],
                start=(sc == 0), stop=(sc == 31))
    aggt_sb = sp.tile([128, 12], F32)
    for db in range(4):
        nc.vector.tensor_copy(out=aggt_sb[:, 3 * db:3 * db + 3],
                              in_=agg_ps[db][:])
    agg_sb = sp.tile([128, 4], F32)
    av = aggt_sb[:].rearrange("p (db three) -> p db three", three=3)
    nc.vector.tensor_scalar_mul(av[:, :, 1:2], av[:, :, 1:2], 1.0 / 64)
    nc.vector.tensor_scalar_mul(av[:, :, 2:3], av[:, :, 2:3], 1.0 / 4096)
    nc.vector.tensor_reduce(out=agg_sb[:], in_=av,
                            axis=mybir.AxisListType.X, op=OP.add)

    s_sb = sp.tile([128, 4], F32)
    nc.vector.tensor_tensor(out=s_sb[:], in0=agg_sb[:],
                            in1=dinv_sb[:, 0:4], op=OP.mult)
    return s_sb, dinv_sb


def _build_kernel_general(ct_bf16=False):
    """b1 != 0 fallback: original kernel, full Wr row-parallel matvec."""
    nc = bacc.Bacc("TRN2", target_bir_lowering=False, debug=False,
                   num_devices=NCORES)

    pk_d = nc.dram_tensor("packed", [128, 96], I32, kind="ExternalInput")
    ct_dt = BF16 if ct_bf16 else FP8
    ct_d = nc.dram_tensor("ct", [N, NPC], ct_dt, kind="ExternalInput")
    wb_d = nc.dram_tensor("w1b1", [1, 2 * HID], F32, kind="ExternalInput")
    bias_d = nc.dram_tensor("bias", [1, Y], F32, kind="ExternalInput")
    wr_d = nc.dram_tensor("wr", [8 * NPC, Y], WR_DT, kind="ExternalInput")
    y_d = nc.dram_tensor("y", [1, Y], F32, kind="ExternalOutput")

    with tile.TileContext(nc) as tc:
        with (
            tc.tile_pool(name="small", bufs=1) as sp,
            tc.tile_pool(name="wr", bufs=1) as wp,
            tc.tile_pool(name="psum", bufs=1, space="PSUM") as pp,
        ):
            wbrow = sp.tile([1, 2 * HID], F32)
            nc.sync.dma_start(out=wbrow[:], in_=wb_d[:])
            w1row = wbrow[:, 0:HID]
            b1row = wbrow[:, HID:2 * HID]
            bias_sb = sp.tile([1, Y], F32)
            nc.sync.dma_start(out=bias_sb[:], in_=bias_d[:])

            s_sb, _ = _mp_subgraph(nc, sp, pp, pk_d, ct_d, ct_dt)

            ones_sb = sp.tile([1, 128], F32)
            nc.vector.memset(ones_sb[:], 1.0)
            wb_ps = pp.tile([128, 2 * HID], F32, name="ps4")
            nc.tensor.matmul(out=wb_ps[:, 0:HID], lhsT=ones_sb[:],
                             rhs=w1row, start=True, stop=True)
            nc.tensor.matmul(out=wb_ps[:, HID:2 * HID], lhsT=ones_sb[:],
                             rhs=b1row, start=True, stop=True)
            wb_sb = sp.tile([128, 2 * HID], F32)
            nc.vector.tensor_copy(out=wb_sb[:], in_=wb_ps[:])

            h_sb = sp.tile([128, 4 * HID], BF16)
            for kk in range(HID):
                nc.vector.tensor_scalar(
                    out=h_sb[:, 4 * kk:4 * kk + 4], in0=s_sb[:],
                    scalar1=wb_sb[:, kk:kk + 1],
                    scalar2=wb_sb[:, HID + kk:HID + kk + 1],
                    op0=OP.mult, op1=OP.add)
            nc.vector.tensor_scalar_max(h_sb[:], h_sb[:], 0.0)

            # alias the MP agg banks (aps0-3) and wb bank (ps4): the PSUM
            # pool dedupes tiles by name and only 8 banks exist.  Bias is
            # added during the PSUM->SBUF copy (a DVE preload would be lost:
            # only TensorE matmuls set the has_written accumulate bits).
            y_ps = [pp.tile([1, 512], F32,
                            name=(f"aps{bk}" if bk < 4 else f"ps{bk}"))
                    for bk in range(8)]
            for t in range(32):
                wr_sb = wp.tile([128, Y], WR_DT, name=f"wr{t % 12}")
                nc.sync.dma_start(out=wr_sb[:],
                                  in_=wr_d[128 * t:128 * (t + 1), :])
                kk, c = t // 4, t % 4
                hcol = h_sb[:, 4 * kk + c:4 * kk + c + 1]
                for bk in range(8):
                    nc.tensor.matmul(out=y_ps[bk][:], lhsT=hcol,
                                     rhs=wr_sb[:, 512 * bk:512 * (bk + 1)],
                                     start=(t == 0), stop=(t == 31),
                                     skip_group_check=True)

            y_sb = sp.tile([1, Y], F32)
            for bk in range(8):
                nc.vector.tensor_tensor(
                    out=y_sb[:, 512 * bk:512 * (bk + 1)],
                    in0=y_ps[bk][:],
                    in1=bias_sb[:, 512 * bk:512 * (bk + 1)], op=OP.add)
            nc.sync.dma_start(out=y_d[:], in_=y_sb[:])

    nc.compile()
    return nc


_NC_CACHE = {}


def _get_nc(kind, ct_bf16=False):
    key = (kind, ct_bf16)
    if key not in _NC_CACHE:
        build = _build_kernel_fast if kind == "fast" else _build_kernel_general
        _NC_CACHE[key] = build(ct_bf16)
    return _NC_CACHE[key]


def _graph_prep(x, edge_index):
    """Per-core packed x/indptr arrays and dense count matrices."""
    x = np.ascontiguousarray(x, dtype=np.float32).reshape(N)
    src = np.asarray(edge_index[0], dtype=np.int64)
    dst = np.asarray(edge_index[1], dtype=np.int64)

    indeg = np.bincount(dst, minlength=N)
    indptr = np.zeros(N + 1, dtype=np.int32)
    np.cumsum(indeg, out=indptr[1:])

    packs, cts, any_bf16 = [], [], False
    p = np.arange(128)[:, None]
    for k in range(NCORES):
        rot = (np.arange(32) + 4 * k) % 32          # column rotation
        g = 128 * rot[None, :] + p                  # [128, 32] global node ids

        mask = (dst >= NPC * k) & (dst < NPC * (k + 1))
        ck = np.zeros((NPC, N), dtype=np.float32)
        np.add.at(ck, (dst[mask] - NPC * k, src[mask]), 1.0)
        ck[np.arange(NPC), NPC * k + np.arange(NPC)] += 1.0
        # counts <= 8 are exact in fp8e4m3; fall back to bf16 otherwise
        any_bf16 = any_bf16 or bool(ck.max() > 8)
        srcperm = g.T.reshape(-1)                   # [(sc i)] -> global node
        cts.append((ck, srcperm))

        packed = np.concatenate([
            x[g].astype(np.float32).view(np.int32),
            indptr[g].astype(np.int32),
            indptr[g + 1].astype(np.int32)], axis=1)
        packs.append(np.ascontiguousarray(packed))
    ct_np = ml_dtypes.bfloat16 if any_bf16 else ml_dtypes.float8_e4m3
    cts = [np.ascontiguousarray(ck[:, srcperm].T).astype(ct_np)
           for ck, srcperm in cts]
    return packs, cts, any_bf16


def _host_prep_fast(x, edge_index, W1, b1, Wr, br):
    """Graph layout plus the W1->A weight fold (b1 == 0 only)."""
    packs, cts, any_bf16 = _graph_prep(x, edge_index)
    # fast path ships x and deg+1 (f32) instead of x + indptr pairs
    indeg = np.bincount(np.asarray(edge_index[1], dtype=np.int64),
                        minlength=N)
    degp1 = (indeg + 1).astype(np.float32)
    xf = np.ascontiguousarray(x, dtype=np.float32).reshape(N)
    W1v = np.ascontiguousarray(W1, dtype=np.float32).reshape(HID)
    Wr3 = np.ascontiguousarray(Wr, dtype=np.float32).reshape(N, HID, Y)

    # A+/-[n, :] = sum_k max(+/-W1_k, 0) * Wr[8n+k, :]
    w1p = np.maximum(W1v, 0.0)
    w1m = np.maximum(-W1v, 0.0)
    ap = np.tensordot(Wr3, w1p, axes=([1], [0]))   # [N, Y]
    am = np.tensordot(Wr3, w1m, axes=([1], [0]))   # [N, Y]
    a_all = np.ascontiguousarray(
        np.concatenate([ap, -am], axis=0)).astype(WR_NP)  # [2N, Y]

    p = np.arange(128)[:, None]
    mask = (p // 16 == np.arange(32)[None, :] % 8).astype(np.float32)
    mask[:, 0:8] = 0.0                              # column 0 is static
    perm = (p % 16 == np.arange(128)[None, :] % 16).astype(np.float32)
    # mask/perm hold exact small values: ship as bf16 (f32-word packed)
    maskv = np.ascontiguousarray(
        mask.astype(ml_dtypes.bfloat16)).view(np.float32)
    permv = np.ascontiguousarray(
        perm.astype(ml_dtypes.bfloat16)).view(np.float32)
    scalevec = np.tile(np.array([[1.0, 1.0 / 64, 1.0 / 4096]],
                                dtype=np.float32), (128, 1))

    t = np.arange(32)[None, :]
    in_maps = []
    for k in range(NCORES):
        rot = (np.arange(32) + 4 * k) % 32          # column rotation
        g = 128 * rot[None, :] + p                  # [128, 32] global node ids
        # wrapbase[m, t] = own-node A-row id of gather slot (m, t)
        wrapbase = (512 * k + 128 * (t // 8) + 16 * (t % 8)
                    + p % 16).astype(np.float32)
        consts = np.concatenate([xf[g], degp1[g], maskv, permv, wrapbase,
                                 scalevec], axis=1)
        packed2 = np.ascontiguousarray(consts).view(np.int32)
        # static column 0: rows = A+ then A- for nodes 512k .. 512k+127
        # (coeffs are relu(s) and relu(-s), both nonnegative)
        a0 = np.concatenate([ap[512 * k:512 * k + 128],
                             am[512 * k:512 * k + 128]], axis=0)
        in_maps.append({
            "packed": np.ascontiguousarray(packed2),
            "ct": cts[k],
            "a_all": a_all,
            "a0": np.ascontiguousarray(a0.astype(WR_NP)),
        })
    return in_maps, any_bf16


def _host_prep_general(x, edge_index, W1, b1, Wr, br):
    packs, cts, any_bf16 = _graph_prep(x, edge_index)
    W1v = np.ascontiguousarray(W1, dtype=np.float32).reshape(1, HID)
    b1v = np.ascontiguousarray(b1, dtype=np.float32).reshape(1, HID)
    brv = np.ascontiguousarray(br, dtype=np.float32).reshape(1, Y)
    Wr3 = np.ascontiguousarray(Wr, dtype=np.float32).reshape(N, HID, Y)

    in_maps = []
    for k in range(NCORES):
        wr_core = np.ascontiguousarray(
            Wr3[NPC * k:NPC * (k + 1)].transpose(1, 0, 2).reshape(8 * NPC, Y),
            dtype=np.float32).astype(WR_NP)
        in_maps.append({
            "packed": packs[k],
            "ct": cts[k],
            "w1b1": np.concatenate([W1v, b1v], axis=1),
            "bias": brv if k == 0 else np.zeros((1, Y), dtype=np.float32),
            "wr": wr_core,
        })
    return in_maps, any_bf16


def _run_fast(x, edge_index, W1, b1, Wr, br, _trace):
    in_maps, ct_bf16 = _host_prep_fast(x, edge_index, W1, b1, Wr, br)
    nc = _get_nc("fast", ct_bf16)
    try:
        res = run_bass_kernel_spmd(nc, in_maps, list(range(NCORES)),
                                   trace=_trace)
    except Exception:
        # one retry: recovers from transiently-poisoned device state
        res = run_bass_kernel_spmd(nc, in_maps, list(range(NCORES)),
                                   trace=_trace)
    y = np.asarray(br, dtype=np.float64).reshape(Y).copy()
    for k in range(NCORES):
        for i in range(4):
            yk = np.asarray(res.results[k][f"y{i}"]).astype(np.float64)
            y += yk.reshape(128, 32).T.reshape(Y)
    return y, res


def _run_general(x, edge_index, W1, b1, Wr, br, _trace):
    in_maps, ct_bf16 = _host_prep_general(x, edge_index, W1, b1, Wr, br)
    nc = _get_nc("general", ct_bf16)
    try:
        res = run_bass_kernel_spmd(nc, in_maps, list(range(NCORES)),
                                   trace=_trace)
    except Exception:
        res = run_bass_kernel_spmd(nc, in_maps, list(range(NCORES)),
                                   trace=_trace)
    y = np.zeros(Y, dtype=np.float64)
    for k in range(NCORES):
        y += np.asarray(res.results[k]["y"]).reshape(Y).astype(np.float64)
    return y, res


def kernel(x, edge_index, W1, b1, Wr, br, _trace=False):
    fast = not np.asarray(b1, dtype=np.float32).any()
    y = res = None
    if fast:
        try:
            y, res = _run_fast(x, edge_index, W1, b1, Wr, br, _trace)
        except Exception:
            y = None  # e.g. gather/gpsimd library unavailable: use fallback
    if y is None:
        y, res = _run_general(x, edge_index, W1, b1, Wr, br, _trace)
    out = y.astype(np.float32)
    if _trace:
        return out, res
    return out
